# revision 1
# baseline (speedup 1.0000x reference)
"""Entmax-alpha (bisection reference) Bass kernel for Trainium2, 8-core SPMD.

Problem: out = entmax_bisect(att_scores[4,16,1024,1024], alpha[16]) over last dim.

Algorithm (mathematically equivalent to the reference's 50-step bisection;
both converge to the same root of S(t)=1 at fp32 precision):
  For each row, solve  S(t) = sum_k (s*(x_k - t))_+^p = 1  with s = alpha-1,
  p = 1/s, by Anderson-Bjorck regula falsi on h = ln S (near-linear in t for
  both the p~1 and p>>1 regimes), with the scaling factor clamped to
  [0.5, 1] (raw AB is chaotic near convergence).  7 evaluations total
  (1 bracket-anchor + 6 iterations) reach the fp32 fixed point of the
  reference; the ~9e-6 residual vs the reference is ACT-spline noise.
  Bracket: t in [max-1/s, max-((1/K)^s)/s]  (S>=1 at left, S<=1 at right,
  and S >= 1/K everywhere in the bracket so ln S stays finite).
  Output: y_k = (s*(x_k - t*))^p / S(t*), using the last evaluation.

Device mapping per evaluation (per [128,4x1024] supertile; 5 supertiles stay
SBUF-resident, the other 11 re-stream x from HBM each evaluation — deep
pipelining without group barriers, at 57% of effective HBM bandwidth):
  Pool: u = max(x - t, eps)            (tensor_scalar sub+max, per-row t)
  ACT : L = Ln(s*u)                    (one pass over 4096 free elems)
  ACT : y = Exp(p*L)                   (one pass; p shared within the head)
  DVE : S[4] = row sums                (tensor_reduce over [128,4,1024])
Root updates run on tiny [128,4] state tiles on DVE.  ScalarE (ACT) is the
bottleneck engine at ~87% occupancy; DVE/Pool/DMA all sit at 65-85%.

Sharding: data-parallel over B*H (64 head-blocks) -> 8 blocks per core.
"""

import numpy as np

import concourse.bacc as bacc
import concourse.mybir as mybir
from concourse.tile import TileContext
from concourse.bass_utils import run_bass_kernel_spmd

B, H, Q, K = 4, 16, 1024, 1024
NCORES = 8
BLOCKS = (B * H) // NCORES      # head-blocks per core (8)
import os as _os
R = int(_os.environ.get("RSUB", "4"))  # q-subrows per partition per supertile
ST_ROWS = 128 * R               # rows per supertile (512)
N_ST = BLOCKS * Q // ST_ROWS    # supertiles per core (16)
GROUP = int(_os.environ.get("GROUPN", "8"))   # supertiles per trace chunk
WT_BUFS = int(_os.environ.get("WTBUFS", "6"))   # work-tile pipeline depth
STREAM = _os.environ.get("STREAM", "1") == "1"  # re-stream x from HBM per eval
XT_BUFS = int(_os.environ.get("XTBUFS", "6")) or GROUP
# First RES_N supertiles stay SBUF-resident (loaded once); the rest re-stream
# every evaluation. Cuts 8-core aggregate HBM demand ~28% vs full streaming.
RES_N = int(_os.environ.get("RESN", "0"))
NC = N_ST * R                   # state columns (64)
K_ITERS = int(_os.environ.get("KITERS", "6"))  # root iterations (last one produces output)
REDUCE_VARIANT = _os.environ.get("RVAR", "1") == "1"
CSPLIT = _os.environ.get("CSPLIT", "1") == "1"
CSPLIT_N = int(_os.environ.get("CSPLITN", "3"))
INIT_DVE_CLAMP = _os.environ.get("IDC", "0") == "1"
TTR_RED = _os.environ.get("TTRRED", "0") == "1"
TTR_MAX = _os.environ.get("TTRMAX", "0") == "1"
EPS = 1e-30

AL = mybir.AluOpType
AF = mybir.ActivationFunctionType
F32 = mybir.dt.float32

LAST_RESULT = None              # BassKernelResults of the most recent run


def _build():
    nc = bacc.Bacc(None, target_bir_lowering=False)
    x_in = nc.declare_dram_parameter("x", [BLOCKS * Q, K], F32, isOutput=False)
    cst_in = nc.declare_dram_parameter("cst", [128, 4 * NC], F32, isOutput=False)
    y_out = nc.declare_dram_parameter("y", [BLOCKS * Q, K], F32, isOutput=True)

    with TileContext(nc) as tc:
        with tc.tile_pool(name="state", bufs=1) as stp, \
             tc.tile_pool(name="xgrp", bufs=XT_BUFS) as xgp, \
             tc.tile_pool(name="work", bufs=WT_BUFS) as wpp, \
             tc.tile_pool(name="rdscratch", bufs=2) as rdp:
            v = nc.vector

            cst = stp.tile([128, 4 * NC], F32)
            nc.sync.dma_start(cst[:, :], cst_in[:, :])
            c1 = cst[:, 0 * NC:1 * NC]   # 1/s
            c2 = cst[:, 1 * NC:2 * NC]   # ((1/K)^s)/s
            sC = cst[:, 2 * NC:3 * NC]   # s
            pC = cst[:, 3 * NC:4 * NC]   # p = 1/s

            mx = stp.tile([128, NC], F32)
            Pt = stp.tile([128, NC], F32)   # positive-side endpoint (h>=0)
            Nt = stp.tile([128, NC], F32)   # negative-side endpoint (h<=0)
            hp = stp.tile([128, NC], F32)
            hn = stp.tile([128, NC], F32)
            hx = stp.tile([128, NC], F32)
            Sp = stp.tile([128, NC], F32)
            xs = stp.tile([128, NC], F32)   # current evaluation point
            U8 = mybir.dt.uint8
            mpos = stp.tile([128, NC], U8)
            mneg = stp.tile([128, NC], U8)
            ppos = stp.tile([128, NC], U8)  # prev-iter side bits
            pneg = stp.tile([128, NC], U8)
            tm = stp.tile([128, NC], U8)
            t1 = stp.tile([128, NC], F32)
            t2 = stp.tile([128, NC], F32)
            rS = stp.tile([128, NC], F32)

            v.memset(ppos[:, :], 1)
            v.memset(pneg[:, :], 0)
            v.memset(rS[:, 0:1], 1.0)
            nc.scalar.activation(rS[:, 0:1], rS[:, 0:1], AF.Ln)

            def x_dram_ap(handle, st):
                r0 = st * ST_ROWS
                return handle[r0:r0 + ST_ROWS, :].rearrange(
                    "(j p) k -> p j k", p=128)

            def sb3(tile_ap):
                return tile_ap.rearrange("p (j k) -> p j k", k=K)

            def do_eval(xt, st, t_tile, wt, init=False):
                """wt = Exp(p*Ln(s*max(x - t, eps))), Sp[cols] = row sums."""
                cc = st * R
                for j in range(R):
                    if init and INIT_DVE_CLAMP:
                        clamp_eng = v
                    elif CSPLIT:
                        clamp_eng = nc.gpsimd if j < CSPLIT_N else v
                    else:
                        clamp_eng = nc.gpsimd if REDUCE_VARIANT else v
                    clamp_eng.tensor_scalar(
                        wt[:, j * K:(j + 1) * K], xt[:, j * K:(j + 1) * K],
                        t_tile[:, cc + j:cc + j + 1], EPS,
                        op0=AL.subtract, op1=AL.max)
                nc.scalar.activation(wt[:, :], wt[:, :], AF.Ln,
                                     scale=sC[:, cc:cc + 1])
                if REDUCE_VARIANT:
                    # p is per-head, shared by all subrows: one big Exp.
                    nc.scalar.activation(wt[:, :], wt[:, :], AF.Exp,
                                         scale=pC[:, cc:cc + 1])
                    if TTR_RED:
                        # Per-subrow sum via tensor_tensor_reduce: fold the
                        # two K/2 halves with op0=add while reducing -- half
                        # the DVE cycles of a plain 1x-mode tensor_reduce.
                        for j in range(R):
                            rd = rdp.tile([128, K // 2], F32, name="rd")
                            v.tensor_tensor_reduce(
                                rd[:, :], wt[:, j * K:j * K + K // 2],
                                wt[:, j * K + K // 2:(j + 1) * K],
                                1.0, 0.0, op0=AL.add, op1=AL.add,
                                accum_out=Sp[:, cc + j:cc + j + 1])
                    else:
                        v.tensor_reduce(Sp[:, cc:cc + R], sb3(wt[:, :]),
                                        axis=mybir.AxisListType.X, op=AL.add)
                else:
                    for j in range(R):
                        nc.scalar.activation(
                            wt[:, j * K:(j + 1) * K], wt[:, j * K:(j + 1) * K],
                            AF.Exp, scale=pC[:, cc + j:cc + j + 1],
                            accum_out=Sp[:, cc + j:cc + j + 1])

            st_chunks = [list(range(a, min(a + GROUP, N_ST)))
                         for a in range(0, N_ST, GROUP)]
            for chunk in st_chunks:
                xts = []
                for stl, st in enumerate(chunk):
                    c4 = slice(st * R, st * R + R)
                    if st < RES_N:
                        xt = xgp.tile([128, R * K], F32, name="xr",
                                      tag=f"xr{st}", bufs=1)
                    else:
                        xt = xgp.tile([128, R * K], F32, name="xt")
                    nc.sync.dma_start(sb3(xt[:, :]), x_dram_ap(x_in, st))
                    xts.append(xt)
                    # init: bracket endpoints and h at the left endpoint
                    if TTR_MAX:
                        # row max via TTR fold of the two K/2 halves
                        for j in range(R):
                            rd = rdp.tile([128, K // 2], F32, name="rd")
                            v.tensor_tensor_reduce(
                                rd[:, :], xt[:, j * K:j * K + K // 2],
                                xt[:, j * K + K // 2:(j + 1) * K],
                                1.0, 0.0, op0=AL.max, op1=AL.max,
                                accum_out=mx[:, st * R + j:st * R + j + 1])
                    else:
                        v.tensor_reduce(
                            mx[:, c4],
                            xt[:, :].rearrange("p (j k) -> p j k", k=K),
                            axis=mybir.AxisListType.X, op=AL.max)
                    v.tensor_tensor(Pt[:, c4], mx[:, c4], c1[:, c4],
                                    op=AL.subtract)
                    v.tensor_tensor(Nt[:, c4], mx[:, c4], c2[:, c4],
                                    op=AL.subtract)
                    wt = wpp.tile([128, R * K], F32, name="wt")
                    do_eval(xt, st, Pt, wt, init=True)
                    nc.scalar.activation(hp[:, c4], Sp[:, c4], AF.Ln)
                    v.tensor_scalar_mul(hn[:, c4], hp[:, c4], -1.0)

                for it in range(K_ITERS):
                    last = it == K_ITERS - 1
                    for stl, st in enumerate(chunk):
                        c4 = slice(st * R, st * R + R)
                        # secant point, clipped into the bracket (hoisted
                        # ahead of the eval sub-loop so late supertiles'
                        # dependencies clear before DVE fills with reduces)
                        v.tensor_tensor(t1[:, c4], hn[:, c4], hp[:, c4],
                                        op=AL.subtract)
                        v.tensor_scalar_min(t1[:, c4], t1[:, c4], -1e-30)
                        v.reciprocal(t1[:, c4], t1[:, c4])
                        v.tensor_tensor(t2[:, c4], Nt[:, c4], Pt[:, c4],
                                        op=AL.subtract)
                        v.tensor_tensor(t2[:, c4], t2[:, c4], hn[:, c4],
                                        op=AL.mult)
                        v.tensor_tensor(t2[:, c4], t2[:, c4], t1[:, c4],
                                        op=AL.mult)
                        v.tensor_tensor(xs[:, c4], Nt[:, c4], t2[:, c4],
                                        op=AL.subtract)
                        v.tensor_tensor(t1[:, c4], Pt[:, c4], Nt[:, c4],
                                        op=AL.min)
                        v.tensor_tensor(t2[:, c4], Pt[:, c4], Nt[:, c4],
                                        op=AL.max)
                        v.tensor_tensor(xs[:, c4], xs[:, c4], t1[:, c4],
                                        op=AL.max)
                        v.tensor_tensor(xs[:, c4], xs[:, c4], t2[:, c4],
                                        op=AL.min)

                    for stl, st in enumerate(chunk):
                        c4 = slice(st * R, st * R + R)
                        if STREAM and st >= RES_N:
                            xt_it = xgp.tile([128, R * K], F32, name="xt")
                            nc.sync.dma_start(sb3(xt_it[:, :]),
                                              x_dram_ap(x_in, st))
                        else:
                            xt_it = xts[stl]
                        wt = wpp.tile([128, R * K], F32, name="wt")
                        do_eval(xt_it, st, xs, wt)

                        if not last:
                            nc.scalar.activation(hx[:, c4], Sp[:, c4], AF.Ln)
                            v.tensor_scalar(mpos[:, c4], hx[:, c4], 0.0, None,
                                            op0=AL.is_ge)
                            v.tensor_scalar(mneg[:, c4], hx[:, c4], 0.0, None,
                                            op0=AL.is_lt)
                            # Anderson-Bjorck scaling of the retained side
                            # when stale: fac = clip(1 - hx/h_same, 0.5, 1).
                            # The lower clip keeps the retained h from
                            # collapsing (raw AB is chaotic near convergence).
                            v.tensor_tensor(tm[:, c4], mpos[:, c4],
                                            ppos[:, c4], op=AL.bitwise_and)
                            v.tensor_scalar(t1[:, c4], hp[:, c4], 1e-30, None,
                                            op0=AL.max)
                            v.reciprocal(t1[:, c4], t1[:, c4])
                            v.tensor_tensor(t1[:, c4], hx[:, c4], t1[:, c4],
                                            op=AL.mult)
                            v.tensor_scalar(t1[:, c4], t1[:, c4], -1.0, 1.0,
                                            op0=AL.mult, op1=AL.add)
                            v.tensor_scalar(t1[:, c4], t1[:, c4], 0.5, 1.0,
                                            op0=AL.max, op1=AL.min)
                            v.tensor_tensor(t2[:, c4], hn[:, c4], t1[:, c4],
                                            op=AL.mult)
                            v.copy_predicated(hn[:, c4], tm[:, c4], t2[:, c4])
                            v.tensor_tensor(tm[:, c4], mneg[:, c4],
                                            pneg[:, c4], op=AL.bitwise_and)
                            v.tensor_scalar(t1[:, c4], hn[:, c4], -1e-30, None,
                                            op0=AL.min)
                            v.reciprocal(t1[:, c4], t1[:, c4])
                            v.tensor_tensor(t1[:, c4], hx[:, c4], t1[:, c4],
                                            op=AL.mult)
                            v.tensor_scalar(t1[:, c4], t1[:, c4], -1.0, 1.0,
                                            op0=AL.mult, op1=AL.add)
                            v.tensor_scalar(t1[:, c4], t1[:, c4], 0.5, 1.0,
                                            op0=AL.max, op1=AL.min)
                            v.tensor_tensor(t2[:, c4], hp[:, c4], t1[:, c4],
                                            op=AL.mult)
                            v.copy_predicated(hp[:, c4], tm[:, c4], t2[:, c4])
                            # move the endpoint the new point replaces
                            v.copy_predicated(hp[:, c4], mpos[:, c4], hx[:, c4])
                            v.copy_predicated(Pt[:, c4], mpos[:, c4], xs[:, c4])
                            v.copy_predicated(hn[:, c4], mneg[:, c4], hx[:, c4])
                            v.copy_predicated(Nt[:, c4], mneg[:, c4], xs[:, c4])
                            v.tensor_copy(ppos[:, c4], mpos[:, c4])
                            v.tensor_copy(pneg[:, c4], mneg[:, c4])
                        else:
                            v.reciprocal(rS[:, c4], Sp[:, c4])
                            cc = st * R
                            for j in range(R):
                                v.tensor_scalar_mul(
                                    wt[:, j * K:(j + 1) * K],
                                    wt[:, j * K:(j + 1) * K],
                                    rS[:, cc + j:cc + j + 1])
                            nc.sync.dma_start(x_dram_ap(y_out, st), sb3(wt[:, :]))
    # Our only ACT functions are Ln and Exp. The greedy table-load pass
    # assigns Exp->exp_and_others and Ln->natural_log, forcing a ~2.7us
    # table reload before nearly every ACTIVATE (316 loads). Empty every
    # set except natural_log_exp_and_others (which holds both) so a single
    # table load serves the whole kernel. Positions are preserved because
    # the set id is the index in this dict.
    orig_tables = bacc.get_activation_tables

    def _lnexp_only(arch):
        return {k: (v if k == "natural_log_exp_and_others" else set())
                for k, v in orig_tables(arch).items()}

    bacc.get_activation_tables = _lnexp_only
    try:
        nc.finalize()
    finally:
        bacc.get_activation_tables = orig_tables
    return nc


_NC_CACHE = None


def _get_nc():
    global _NC_CACHE
    if _NC_CACHE is None:
        _NC_CACHE = _build()
    return _NC_CACHE


def kernel(att_scores: np.ndarray, alpha: np.ndarray) -> np.ndarray:
    X = np.ascontiguousarray(np.asarray(att_scores, dtype=np.float32))
    X = X.reshape(B * H, Q, K)
    al = np.asarray(alpha, dtype=np.float64).reshape(H)

    nc = _get_nc()
    in_maps = []
    for c in range(NCORES):
        xc = np.ascontiguousarray(
            X[c * BLOCKS:(c + 1) * BLOCKS].reshape(BLOCKS * Q, K))
        cvec = np.zeros((4, NC), np.float64)
        for st in range(N_ST):
            h = (c * BLOCKS + st // (Q // ST_ROWS)) % H
            s = al[h] - 1.0
            cols = slice(st * R, st * R + R)
            cvec[0, cols] = 1.0 / s
            cvec[1, cols] = ((1.0 / K) ** s) / s
            cvec[2, cols] = s
            cvec[3, cols] = 1.0 / s
        cst = np.tile(cvec.reshape(1, 4 * NC).astype(np.float32), (128, 1))
        in_maps.append({"x": xc, "cst": cst})

    res = run_bass_kernel_spmd(nc, in_maps, core_ids=list(range(NCORES)))
    global LAST_RESULT
    LAST_RESULT = res
    outs = [np.asarray(res.results[c]["y"]) for c in range(NCORES)]
    return np.concatenate(outs, axis=0).reshape(B, H, Q, K).astype(np.float32)



# revision 10
# speedup vs baseline: 1.0971x; 1.0971x over previous
"""Entmax-alpha Bass kernel for Trainium2, 8-core SPMD — sketch+Newton design.

Problem: out = entmax_bisect(att_scores[4,16,1024,1024], alpha[16]) over last
dim; graded metric absmax_rel < 2e-2 (this build reaches ~2e-3).

Algorithm (3 full-data evaluations instead of the reference's 50):
  1. SKETCH: per row, take the 16 chunk-maxes (chunks of 64). Running the
     entmax bisection on this 16-value sketch (6 iters, tiny state tiles)
     nearly exactly solves the PEAKED rows (the ones plain Newton struggles
     with, since S(t) has a kink where elements cross the support threshold),
     and lands q99 ~0.07 from the true tau overall.  Cost: one bf16 cast-load
     pass + cheap [128,512]-tile iterations.
  2. NEWTON: one full evaluation at t0 computing S0 = sum w and
     m1 = sum (s*u)^(p-1) (an extra Exp over the same Ln output), giving the
     exact local derivative  dlnS/dt = -p*s*m1/S  ->  t1.
  3. OUTPUT + CORRECTION: evaluate w1, S1 at t1; a secant step from
     (t0,h0),(t1,h1) predicts t3; first-order in-place correction
     y ~ w1 + p*s*(t1-t3)*v1  (v1 = (s*u1)^(p-1), again reusing Ln output),
     then normalize.  The correction makes the result second-order in the
     remaining tau error (absmax ~2e-3 vs reference).

Per-core device mapping (16 supertiles of [128 part x 4 subrows x 1024]):
  ACT: 6 full passes (Ln,Exp(p),Exp(p-1)) x 2 evals + sketch Ln/Exp
  DVE: chunk-max reduce, sketch tiles, TTR row-sums, state math, normalize
  Pool(GPSIMD): the 4 clamps/supertile x 2 evals, corr fused mul-add (STT)
  DMA: bf16 cast-load (sketch pass), fp32 load (eval pass), fp32 store
Sharding: data-parallel over B*H (64 head-blocks) -> 8 blocks per core; all
per-head constants arrive via the per-core cst input (single SPMD NEFF).
"""

import numpy as np

import concourse.bacc as bacc
import concourse.mybir as mybir
from concourse.tile import TileContext
from concourse.bass_utils import run_bass_kernel_spmd

B, H, Q, K = 4, 16, 1024, 1024
NCORES = 8
BLOCKS = (B * H) // NCORES      # head-blocks per core (8)
R = 4                           # q-subrows per partition per supertile
ST_ROWS = 128 * R               # rows per supertile (512)
N_ST = BLOCKS * Q // ST_ROWS    # supertiles per core (16)
NC = N_ST * R                   # state columns (64)
NCH = 16                        # sketch chunks per row
CHW = K // NCH                  # chunk width (64)
import os as _os
SK_ITERS = int(_os.environ.get("SKITERS", "6"))
DBG_NC = int(_os.environ.get("DBG_NC", str(-1)))      # phaseC sts (-1=all)
DBG_SKETCH = _os.environ.get("DBG_SKETCH", "1") == "1"
DBG_CSTAGE = int(_os.environ.get("DBG_CSTAGE", "99"))
DBG_DUMP = _os.environ.get("DBG_DUMP", "0") == "1" 
SK_GROUPS = 4                   # supertiles per sketch group = N_ST//SK_GROUPS
GSTS = N_ST // SK_GROUPS        # sts per group (4)
GW = GSTS * R * NCH             # sketch tile width per group (256)
GS = GSTS * R                   # state cols per group (16)
EPS = 1e-30

AL = mybir.AluOpType
AF = mybir.ActivationFunctionType
F32 = mybir.dt.float32
BF16 = mybir.dt.bfloat16
U8 = mybir.dt.uint8

# cst layout (fp32, replicated across 128 partitions):
#   [0:1024)            P-tiles: p per sketch column, 4 groups x 256
#   [1024:1088)         DM0 per (st,j): 1 - (1/K)^s
#   [1088:1104)         sC   per st: s
#   [1104:1120)         pC   per st: p
#   [1120:1136)         pm1C per st: p-1
#   [1136:1152)         isC  per st: 1/s
#   [1152:1168)         ipsC per st: 1/(p*s)
#   [1168:1184)         psC  per st: p*s
#   [1184:1200)         ntcC per st: (1/K)^s
CST_W = 1200

LAST_RESULT = None


def _build():
    nc = bacc.Bacc(None, target_bir_lowering=False)
    x_in = nc.declare_dram_parameter("x", [BLOCKS * Q, K], F32, isOutput=False)
    cst_in = nc.declare_dram_parameter("cst", [128, CST_W], F32, isOutput=False)
    y_out = nc.declare_dram_parameter("y", [BLOCKS * Q, K], F32, isOutput=True)
    dbg_out = (nc.declare_dram_parameter("dbg", [128, 12 * NC], F32,
                                         isOutput=True) if DBG_DUMP else None)

    def x_dram_ap(handle, st):
        r0 = st * ST_ROWS
        return handle[r0:r0 + ST_ROWS, :].rearrange("(j p) k -> p j k", p=128)

    def sb3(tile_ap):
        return tile_ap.rearrange("p (j k) -> p j k", k=K)

    with TileContext(nc) as tc:
        with tc.tile_pool(name="state", bufs=1) as stp, \
             tc.tile_pool(name="xa", bufs=2) as pxa, \
             tc.tile_pool(name="xt", bufs=2) as pxt, \
             tc.tile_pool(name="uu", bufs=4) as puu, \
             tc.tile_pool(name="wb", bufs=5) as pwb, \
             tc.tile_pool(name="rd", bufs=2) as prd, \
             tc.tile_pool(name="skw", bufs=2) as psk:
            v = nc.vector

            cst = stp.tile([128, CST_W], F32)
            nc.sync.dma_start(cst[:, :], cst_in[:, :])
            PT = cst[:, 0:1024]
            DM0 = cst[:, 1024:1088]
            sC = cst[:, 1088:1104]
            pC = cst[:, 1104:1120]
            pm1C = cst[:, 1120:1136]
            isC = cst[:, 1136:1152]
            ipsC = cst[:, 1152:1168]
            psC = cst[:, 1168:1184]
            ntcC = cst[:, 1184:1200]

            # whole-core state tiles [128, NC]
            CM = stp.tile([128, NC * NCH], F32)     # s-scaled chunk maxes
            MXS = stp.tile([128, NC], F32)          # row max (s-domain)
            T0 = stp.tile([128, NC], F32)           # t0 (x-domain)
            T1 = stp.tile([128, NC], F32)
            LOX = stp.tile([128, NC], F32)
            HIX = stp.tile([128, NC], F32)
            H0 = stp.tile([128, NC], F32)
            S0 = stp.tile([128, NC], F32)
            M1 = stp.tile([128, NC], F32)
            S1 = stp.tile([128, NC], F32)
            M11 = stp.tile([128, NC], F32)
            SP = stp.tile([128, NC], F32)
            CCORR = stp.tile([128, NC], F32)        # p*s*(t1-t3)
            t1a = stp.tile([128, NC], F32)
            t1b = stp.tile([128, NC], F32)
            # sketch state per group
            slo = stp.tile([128, NC], F32)
            sdm = stp.tile([128, NC], F32)
            stm = stp.tile([128, NC], F32)
            smask = stp.tile([128, NC], U8)
            ssum = stp.tile([128, NC], F32)
            ZB = stp.tile([128, K], BF16)
            v.memset(ZB[:, :], 0.0)

            def phaseA(st):
                """bf16 cast-load, chunk maxes (s-scaled), row max."""
                g, gl = st // GSTS, st % GSTS
                xa = pxa.tile([128, R * K], BF16, name="xa")
                nc.gpsimd.dma_start(sb3(xa[:, :]), x_dram_ap(x_in, st))
                cmsl = CM[:, st * R * NCH:(st + 1) * R * NCH]
                v.tensor_reduce(
                    cmsl.rearrange("p (j c) -> p j c", c=NCH),
                    xa[:, :].rearrange("p (j c k) -> p j c k", c=NCH, k=CHW),
                    axis=mybir.AxisListType.X, op=AL.max)
                # scale to s-domain in place
                v.tensor_scalar(cmsl, cmsl, sC[:, st:st + 1], None, op0=AL.mult)
                c4 = slice(st * R, st * R + R)
                v.tensor_reduce(MXS[:, c4],
                                cmsl.rearrange("p (j c) -> p j c", c=NCH),
                                axis=mybir.AxisListType.X, op=AL.max)
                # brackets in x units: lox=(mxs-1)/s, hix=(mxs-ntc)/s
                v.tensor_scalar(t1a[:, c4], MXS[:, c4], 1.0, None,
                                op0=AL.subtract)
                v.tensor_scalar(LOX[:, c4], t1a[:, c4], isC[:, st:st + 1],
                                None, op0=AL.mult)
                v.tensor_scalar(t1b[:, c4], MXS[:, c4], ntcC[:, st:st + 1],
                                None, op0=AL.subtract)
                v.tensor_scalar(HIX[:, c4], t1b[:, c4], isC[:, st:st + 1],
                                None, op0=AL.mult)

            def sketch(g):
                """bisection on the s-domain chunk maxes of group g."""
                gc = slice(g * GS, (g + 1) * GS)            # state cols
                gw = slice(g * GS * NCH, (g + 1) * GS * NCH)  # sketch cols
                cm = CM[:, gw]
                v.tensor_scalar(slo[:, gc], MXS[:, gc], 1.0, None,
                                op0=AL.subtract)
                v.tensor_copy(sdm[:, gc], DM0[:, gc])
                if not DBG_SKETCH:
                    v.tensor_tensor(stm[:, gc], slo[:, gc], sdm[:, gc],
                                    op=AL.add)
                for it in range(SK_ITERS if DBG_SKETCH else 0):
                    v.tensor_scalar(sdm[:, gc], sdm[:, gc], 0.5, None,
                                    op0=AL.mult)
                    v.tensor_tensor(stm[:, gc], slo[:, gc], sdm[:, gc],
                                    op=AL.add)
                    wt = psk.tile([128, GW], F32, name="skw")
                    # u = max(cm - t, eps); t broadcast along chunk dim
                    v.tensor_tensor(
                        wt[:, :].rearrange("p (s c) -> p s c", c=NCH),
                        cm.rearrange("p (s c) -> p s c", c=NCH),
                        stm[:, gc].rearrange("p (s o) -> p s o", o=1)
                        .broadcast_to((128, GS, NCH)),
                        op=AL.subtract)
                    v.tensor_scalar(wt[:, :], wt[:, :], EPS, None, op0=AL.max)
                    nc.scalar.activation(wt[:, :], wt[:, :], AF.Ln)
                    v.tensor_tensor(wt[:, :], wt[:, :], PT[:, gw], op=AL.mult)
                    nc.scalar.activation(wt[:, :], wt[:, :], AF.Exp)
                    v.tensor_reduce(ssum[:, gc],
                                    wt[:, :].rearrange("p (s c) -> p s c",
                                                       c=NCH),
                                    axis=mybir.AxisListType.X, op=AL.add)
                    v.tensor_scalar(smask[:, gc], ssum[:, gc], 1.0, None,
                                    op0=AL.is_ge)
                    v.copy_predicated(slo[:, gc], smask[:, gc], stm[:, gc])
                v.tensor_tensor(stm[:, gc], slo[:, gc], sdm[:, gc], op=AL.add)
                # to x-domain per st: t0 = t0s/s, clipped
                for stl in range(GSTS):
                    st = g * GSTS + stl
                    c4 = slice(st * R, st * R + R)
                    v.tensor_scalar(T0[:, c4], stm[:, c4], isC[:, st:st + 1],
                                    None, op0=AL.mult)
                v.tensor_tensor(T0[:, gc], T0[:, gc], LOX[:, gc], op=AL.max)
                v.tensor_tensor(T0[:, gc], T0[:, gc], HIX[:, gc], op=AL.min)

            def clamp(dst, xt, tcol_tile, st):
                cc = st * R
                for j in range(R):
                    nc.gpsimd.tensor_scalar(
                        dst[:, j * K:(j + 1) * K], xt[:, j * K:(j + 1) * K],
                        tcol_tile[:, cc + j:cc + j + 1], EPS,
                        op0=AL.subtract, op1=AL.max)

            def row_sums(wt, dst, st):
                # per-subrow sums via bf16 STT-with-accum (TTR is broken on
                # this runtime; STT at 2x costs the same)
                cc = st * R
                for j in range(R):
                    rdt = prd.tile([128, K], BF16, name="rd")
                    v.scalar_tensor_tensor(
                        rdt[:, :], wt[:, j * K:(j + 1) * K], 1.0, ZB[:, :],
                        op0=AL.bypass, op1=AL.add,
                        accum_out=dst[:, cc + j:cc + j + 1])

            def phaseC(st):
                c4 = slice(st * R, st * R + R)
                STG = DBG_CSTAGE
                sc1 = sC[:, st:st + 1]
                pc1 = pC[:, st:st + 1]
                pm11 = pm1C[:, st:st + 1]
                xt = pxt.tile([128, R * K], F32, name="xt")
                nc.sync.dma_start(sb3(xt[:, :]), x_dram_ap(x_in, st))
                # ---- eval0
                u0 = puu.tile([128, R * K], F32, name="uu")
                clamp(u0, xt, T0, st)
                if STG <= 1:
                    return
                nc.scalar.activation(u0[:, :], u0[:, :], AF.Ln, scale=sc1)
                if STG <= 2:
                    return
                w0 = pwb.tile([128, R * K], BF16, name="wb")
                nc.scalar.activation(w0[:, :], u0[:, :], AF.Exp, scale=pc1)
                if STG <= 3:
                    return
                row_sums(w0, S0, st)
                if STG <= 4:
                    return
                v0 = pwb.tile([128, R * K], BF16, name="wb")
                nc.scalar.activation(v0[:, :], u0[:, :], AF.Exp, scale=pm11)
                row_sums(v0, M1, st)
                if STG <= 5:
                    return
                # ---- Newton: t1 = clip(t0 + h0*S0/(p*s*m1))
                nc.scalar.activation(H0[:, c4], S0[:, c4], AF.Ln)
                v.tensor_tensor(t1a[:, c4], H0[:, c4], S0[:, c4], op=AL.mult)
                v.reciprocal(t1b[:, c4], M1[:, c4])
                v.tensor_tensor(t1a[:, c4], t1a[:, c4], t1b[:, c4],
                                op=AL.mult)
                v.tensor_scalar(t1a[:, c4], t1a[:, c4], ipsC[:, st:st + 1],
                                None, op0=AL.mult)
                v.tensor_tensor(T1[:, c4], T0[:, c4], t1a[:, c4], op=AL.add)
                v.tensor_tensor(T1[:, c4], T1[:, c4], LOX[:, c4], op=AL.max)
                v.tensor_tensor(T1[:, c4], T1[:, c4], HIX[:, c4], op=AL.min)
                if STG <= 6:
                    return
                # ---- eval1
                u1 = puu.tile([128, R * K], F32, name="uu")
                clamp(u1, xt, T1, st)
                nc.scalar.activation(u1[:, :], u1[:, :], AF.Ln, scale=sc1)
                w1 = pwb.tile([128, R * K], BF16, name="wb")
                nc.scalar.activation(w1[:, :], u1[:, :], AF.Exp, scale=pc1)
                row_sums(w1, S1, st)
                if STG <= 7:
                    return
                v1 = pwb.tile([128, R * K], BF16, name="wb")
                nc.scalar.activation(v1[:, :], u1[:, :], AF.Exp, scale=pm11)
                # ---- Newton at t1 -> t3; corr coeff c = p*s*(t1-t3).
                # (A secant here is numerically fragile: on converged rows
                # bf16-rounded w makes S1 == S0 exactly -> 0/0.)
                row_sums(v1, M11, st)
                nc.scalar.activation(t1b[:, c4], S1[:, c4], AF.Ln)  # h1
                v.tensor_tensor(t1a[:, c4], t1b[:, c4], S1[:, c4],
                                op=AL.mult)                          # h1*S1
                v.reciprocal(t1b[:, c4], M11[:, c4])
                v.tensor_tensor(t1a[:, c4], t1a[:, c4], t1b[:, c4],
                                op=AL.mult)
                v.tensor_scalar(t1a[:, c4], t1a[:, c4], ipsC[:, st:st + 1],
                                None, op0=AL.mult)                   # t3-t1
                v.tensor_tensor(t1b[:, c4], T1[:, c4], t1a[:, c4],
                                op=AL.add)                           # t3 raw
                v.tensor_tensor(t1b[:, c4], t1b[:, c4], LOX[:, c4], op=AL.max)
                v.tensor_tensor(t1b[:, c4], t1b[:, c4], HIX[:, c4], op=AL.min)
                v.tensor_tensor(t1a[:, c4], T1[:, c4], t1b[:, c4],
                                op=AL.subtract)                      # t1-t3
                v.tensor_scalar(CCORR[:, c4], t1a[:, c4], psC[:, st:st + 1],
                                None, op0=AL.mult)
                if STG <= 8:
                    return
                # ---- y' = w1 + c*v1 (DVE fused STT, bf16 2x) + S' accum
                yp = pwb.tile([128, R * K], BF16, name="wb")
                cc = st * R
                for j in range(R):
                    v.scalar_tensor_tensor(
                        yp[:, j * K:(j + 1) * K], v1[:, j * K:(j + 1) * K],
                        CCORR[:, cc + j:cc + j + 1],
                        w1[:, j * K:(j + 1) * K],
                        op0=AL.mult, op1=AL.add,
                        accum_out=SP[:, cc + j:cc + j + 1])
                if STG <= 9:
                    return
                v.reciprocal(t1a[:, c4], SP[:, c4])
                yt = puu.tile([128, R * K], F32, name="uu")
                for j in range(R):
                    v.tensor_scalar(yt[:, j * K:(j + 1) * K],
                                    yp[:, j * K:(j + 1) * K],
                                    t1a[:, cc + j:cc + j + 1], None,
                                    op0=AL.mult)
                nc.sync.dma_start(x_dram_ap(y_out, st), sb3(yt[:, :]))

            # ---------- schedule ----------
            ncl = N_ST if DBG_NC < 0 else DBG_NC
            def dump_state():
                if dbg_out is None:
                    return
                for i, t in enumerate((T0, T1, S0, M1, S1, SP, CCORR, MXS,
                                       LOX, HIX, H0, stm)):
                    nc.sync.dma_start(dbg_out[:, i * NC:(i + 1) * NC], t[:, :])
            for st in range(GSTS):
                phaseA(st)
            sketch(0)
            for st in range(GSTS, 2 * GSTS):
                phaseA(st)
            for st in range(0, min(2, ncl)):
                phaseC(st)
            sketch(1)
            for st in range(2 * GSTS, 3 * GSTS):
                phaseA(st)
            for st in range(2, min(5, ncl)):
                phaseC(st)
            sketch(2)
            for st in range(3 * GSTS, 4 * GSTS):
                phaseA(st)
            for st in range(5, min(8, ncl)):
                phaseC(st)
            sketch(3)
            for st in range(8, max(8, ncl)):
                phaseC(st)
            dump_state()

    # Only Ln/Exp are used: keep the one table set holding both so no
    # mid-kernel ACT table reloads are scheduled.
    orig_tables = bacc.get_activation_tables

    def _lnexp_only(arch):
        return {k: (vv if k == "natural_log_exp_and_others" else set())
                for k, vv in orig_tables(arch).items()}

    bacc.get_activation_tables = _lnexp_only
    try:
        nc.finalize()
    finally:
        bacc.get_activation_tables = orig_tables
    return nc


_NC_CACHE = None


def _get_nc():
    global _NC_CACHE
    if _NC_CACHE is None:
        _NC_CACHE = _build()
    return _NC_CACHE


def _make_cst(al, core):
    """Per-core constant table [128, CST_W] fp32."""
    c = np.zeros(CST_W, np.float64)
    for st in range(N_ST):
        h = (core * BLOCKS + st // (Q // ST_ROWS)) % H
        s = al[h] - 1.0
        p = 1.0 / s
        g, gl = st // GSTS, st % GSTS
        # P-tile columns for this st
        base = g * GW + gl * R * NCH
        c[base:base + R * NCH] = p
        c[1024 + st * R:1024 + st * R + R] = 1.0 - (1.0 / K) ** s
        c[1088 + st] = s
        c[1104 + st] = p
        c[1120 + st] = p - 1.0
        c[1136 + st] = 1.0 / s
        c[1152 + st] = 1.0 / (p * s)
        c[1168 + st] = p * s
        c[1184 + st] = (1.0 / K) ** s
    return np.tile(c.astype(np.float32)[None, :], (128, 1))


def kernel(att_scores: np.ndarray, alpha: np.ndarray) -> np.ndarray:
    X = np.ascontiguousarray(np.asarray(att_scores, dtype=np.float32))
    X = X.reshape(B * H, Q, K)
    al = np.asarray(alpha, dtype=np.float64).reshape(H)

    nc = _get_nc()
    in_maps = []
    for c in range(NCORES):
        xc = np.ascontiguousarray(
            X[c * BLOCKS:(c + 1) * BLOCKS].reshape(BLOCKS * Q, K))
        in_maps.append({"x": xc, "cst": _make_cst(al, c)})

    res = run_bass_kernel_spmd(nc, in_maps, core_ids=list(range(NCORES)))
    global LAST_RESULT
    LAST_RESULT = res
    outs = [np.asarray(res.results[c]["y"]) for c in range(NCORES)]
    return np.concatenate(outs, axis=0).reshape(B, H, Q, K).astype(np.float32)


# revision 12
# speedup vs baseline: 1.3583x; 1.2381x over previous
"""Entmax-alpha Bass kernel for Trainium2, 8-core SPMD — sketch+Newton design.

Problem: out = entmax_bisect(att_scores[4,16,1024,1024], alpha[16]) over last
dim; graded metric absmax_rel < 2e-2 (this build reaches ~2e-3).

Algorithm (3 full-data evaluations instead of the reference's 50):
  1. SKETCH: per row, take the 16 chunk-maxes (chunks of 64). Running the
     entmax bisection on this 16-value sketch (6 iters, tiny state tiles)
     nearly exactly solves the PEAKED rows (the ones plain Newton struggles
     with, since S(t) has a kink where elements cross the support threshold),
     and lands q99 ~0.07 from the true tau overall.  Cost: one bf16 cast-load
     pass + cheap [128,512]-tile iterations.
  2. NEWTON: one full evaluation at t0 computing S0 = sum w and
     m1 = sum (s*u)^(p-1) (an extra Exp over the same Ln output), giving the
     exact local derivative  dlnS/dt = -p*s*m1/S  ->  t1.
  3. OUTPUT + CORRECTION: evaluate w1, S1 at t1; a secant step from
     (t0,h0),(t1,h1) predicts t3; first-order in-place correction
     y ~ w1 + p*s*(t1-t3)*v1  (v1 = (s*u1)^(p-1), again reusing Ln output),
     then normalize.  The correction makes the result second-order in the
     remaining tau error (absmax ~2e-3 vs reference).

Per-core device mapping (16 supertiles of [128 part x 4 subrows x 1024]):
  ACT: 6 full passes (Ln,Exp(p),Exp(p-1)) x 2 evals + sketch Ln/Exp
  DVE: chunk-max reduce, sketch tiles, TTR row-sums, state math, normalize
  Pool(GPSIMD): the 4 clamps/supertile x 2 evals, corr fused mul-add (STT)
  DMA: bf16 cast-load (sketch pass), fp32 load (eval pass), fp32 store
Sharding: data-parallel over B*H (64 head-blocks) -> 8 blocks per core; all
per-head constants arrive via the per-core cst input (single SPMD NEFF).
"""

import numpy as np

import concourse.bacc as bacc
import concourse.mybir as mybir
from concourse.tile import TileContext
from concourse.bass_utils import run_bass_kernel_spmd

B, H, Q, K = 4, 16, 1024, 1024
NCORES = 8
BLOCKS = (B * H) // NCORES      # head-blocks per core (8)
R = 4                           # q-subrows per partition per supertile
ST_ROWS = 128 * R               # rows per supertile (512)
N_ST = BLOCKS * Q // ST_ROWS    # supertiles per core (16)
NC = N_ST * R                   # state columns (64)
NCH = 16                        # sketch chunks per row
CHW = K // NCH                  # chunk width (64)
import os as _os
SK_ITERS = int(_os.environ.get("SKITERS", "6"))
DBG_NC = int(_os.environ.get("DBG_NC", str(-1)))      # phaseC sts (-1=all)
DBG_SKETCH = _os.environ.get("DBG_SKETCH", "1") == "1"
DBG_CSTAGE = int(_os.environ.get("DBG_CSTAGE", "99"))
DBG_DUMP = _os.environ.get("DBG_DUMP", "0") == "1" 
SK_GROUPS = 4                   # supertiles per sketch group = N_ST//SK_GROUPS
GSTS = N_ST // SK_GROUPS        # sts per group (4)
GW = GSTS * R * NCH             # sketch tile width per group (256)
GS = GSTS * R                   # state cols per group (16)
EPS = 1e-30

AL = mybir.AluOpType
AF = mybir.ActivationFunctionType
F32 = mybir.dt.float32
BF16 = mybir.dt.bfloat16
U8 = mybir.dt.uint8

# cst layout (fp32, replicated across 128 partitions):
#   [0:1024)            P-tiles: p per sketch column, 4 groups x 256
#   [1024:1088)         DM0 per (st,j): 1 - (1/K)^s
#   [1088:1104)         sC   per st: s
#   [1104:1120)         pC   per st: p
#   [1120:1136)         pm1C per st: p-1
#   [1136:1152)         isC  per st: 1/s
#   [1152:1168)         ipsC per st: 1/(p*s)
#   [1168:1184)         psC  per st: p*s
#   [1184:1200)         ntcC per st: (1/K)^s
CST_W = 1200

LAST_RESULT = None


def _build():
    nc = bacc.Bacc(None, target_bir_lowering=False)
    x_in = nc.declare_dram_parameter("x", [BLOCKS * Q, K], F32, isOutput=False)
    cst_in = nc.declare_dram_parameter("cst", [128, CST_W], F32, isOutput=False)
    y_out = nc.declare_dram_parameter("y", [BLOCKS * Q, K], F32, isOutput=True)
    dbg_out = (nc.declare_dram_parameter("dbg", [128, 12 * NC], F32,
                                         isOutput=True) if DBG_DUMP else None)

    def x_dram_ap(handle, st):
        r0 = st * ST_ROWS
        return handle[r0:r0 + ST_ROWS, :].rearrange("(j p) k -> p j k", p=128)

    def sb3(tile_ap):
        return tile_ap.rearrange("p (j k) -> p j k", k=K)

    with TileContext(nc) as tc:
        with tc.tile_pool(name="state", bufs=1) as stp, \
             tc.tile_pool(name="xa", bufs=2) as pxa, \
             tc.tile_pool(name="xt", bufs=2) as pxt, \
             tc.tile_pool(name="uu", bufs=4) as puu, \
             tc.tile_pool(name="wb", bufs=5) as pwb, \
             tc.tile_pool(name="rd", bufs=2) as prd, \
             tc.tile_pool(name="skw", bufs=2) as psk:
            v = nc.vector

            cst = stp.tile([128, CST_W], F32)
            nc.sync.dma_start(cst[:, :], cst_in[:, :])
            PT = cst[:, 0:1024]
            DM0 = cst[:, 1024:1088]
            sC = cst[:, 1088:1104]
            pC = cst[:, 1104:1120]
            pm1C = cst[:, 1120:1136]
            isC = cst[:, 1136:1152]
            ipsC = cst[:, 1152:1168]
            psC = cst[:, 1168:1184]
            ntcC = cst[:, 1184:1200]

            # whole-core state tiles [128, NC]
            CM = stp.tile([128, NC * NCH], F32)     # s-scaled chunk maxes
            MXS = stp.tile([128, NC], F32)          # row max (s-domain)
            T0 = stp.tile([128, NC], F32)           # t0 (x-domain)
            T1 = stp.tile([128, NC], F32)
            LOX = stp.tile([128, NC], F32)
            HIX = stp.tile([128, NC], F32)
            H0 = stp.tile([128, NC], F32)
            S0 = stp.tile([128, NC], F32)
            M1 = stp.tile([128, NC], F32)
            S1 = stp.tile([128, NC], F32)
            M11 = stp.tile([128, NC], F32)
            SP = stp.tile([128, NC], F32)
            CCORR = stp.tile([128, NC], F32)        # p*s*(t1-t3)
            t1a = stp.tile([128, NC], F32)
            t1b = stp.tile([128, NC], F32)
            # sketch state per group
            slo = stp.tile([128, NC], F32)
            sdm = stp.tile([128, NC], F32)
            stm = stp.tile([128, NC], F32)
            smask = stp.tile([128, NC], U8)
            ssum = stp.tile([128, NC], F32)
            ZB = stp.tile([128, K], BF16)
            v.memset(ZB[:, :], 0.0)

            def phaseA(st):
                """bf16 cast-load, chunk maxes (s-scaled), row max."""
                g, gl = st // GSTS, st % GSTS
                xa = pxa.tile([128, R * K], BF16, name="xa")
                nc.gpsimd.dma_start(sb3(xa[:, :]), x_dram_ap(x_in, st))
                cmsl = CM[:, st * R * NCH:(st + 1) * R * NCH]
                v.tensor_reduce(
                    cmsl.rearrange("p (j c) -> p j c", c=NCH),
                    xa[:, :].rearrange("p (j c k) -> p j c k", c=NCH, k=CHW),
                    axis=mybir.AxisListType.X, op=AL.max)
                # scale to s-domain in place
                v.tensor_scalar(cmsl, cmsl, sC[:, st:st + 1], None, op0=AL.mult)
                c4 = slice(st * R, st * R + R)
                v.tensor_reduce(MXS[:, c4],
                                cmsl.rearrange("p (j c) -> p j c", c=NCH),
                                axis=mybir.AxisListType.X, op=AL.max)
                # brackets in x units: lox=(mxs-1)/s, hix=(mxs-ntc)/s
                v.tensor_scalar(t1a[:, c4], MXS[:, c4], 1.0, None,
                                op0=AL.subtract)
                v.tensor_scalar(LOX[:, c4], t1a[:, c4], isC[:, st:st + 1],
                                None, op0=AL.mult)
                v.tensor_scalar(t1b[:, c4], MXS[:, c4], ntcC[:, st:st + 1],
                                None, op0=AL.subtract)
                v.tensor_scalar(HIX[:, c4], t1b[:, c4], isC[:, st:st + 1],
                                None, op0=AL.mult)

            def sketch(g):
                """bisection on the s-domain chunk maxes of group g."""
                gc = slice(g * GS, (g + 1) * GS)            # state cols
                gw = slice(g * GS * NCH, (g + 1) * GS * NCH)  # sketch cols
                cm = CM[:, gw]
                v.tensor_scalar(slo[:, gc], MXS[:, gc], 1.0, None,
                                op0=AL.subtract)
                v.tensor_copy(sdm[:, gc], DM0[:, gc])
                if not DBG_SKETCH:
                    v.tensor_tensor(stm[:, gc], slo[:, gc], sdm[:, gc],
                                    op=AL.add)
                for it in range(SK_ITERS if DBG_SKETCH else 0):
                    v.tensor_scalar(sdm[:, gc], sdm[:, gc], 0.5, None,
                                    op0=AL.mult)
                    v.tensor_tensor(stm[:, gc], slo[:, gc], sdm[:, gc],
                                    op=AL.add)
                    wt = psk.tile([128, GW], F32, name="skw")
                    # u = max(cm - t, eps); t broadcast along chunk dim
                    v.tensor_tensor(
                        wt[:, :].rearrange("p (s c) -> p s c", c=NCH),
                        cm.rearrange("p (s c) -> p s c", c=NCH),
                        stm[:, gc].rearrange("p (s o) -> p s o", o=1)
                        .broadcast_to((128, GS, NCH)),
                        op=AL.subtract)
                    v.tensor_scalar(wt[:, :], wt[:, :], EPS, None, op0=AL.max)
                    nc.scalar.activation(wt[:, :], wt[:, :], AF.Ln)
                    v.tensor_tensor(wt[:, :], wt[:, :], PT[:, gw], op=AL.mult)
                    nc.scalar.activation(wt[:, :], wt[:, :], AF.Exp)
                    v.tensor_reduce(ssum[:, gc],
                                    wt[:, :].rearrange("p (s c) -> p s c",
                                                       c=NCH),
                                    axis=mybir.AxisListType.X, op=AL.add)
                    v.tensor_scalar(smask[:, gc], ssum[:, gc], 1.0, None,
                                    op0=AL.is_ge)
                    v.copy_predicated(slo[:, gc], smask[:, gc], stm[:, gc])
                v.tensor_tensor(stm[:, gc], slo[:, gc], sdm[:, gc], op=AL.add)
                # to x-domain per st: t0 = t0s/s, clipped
                for stl in range(GSTS):
                    st = g * GSTS + stl
                    c4 = slice(st * R, st * R + R)
                    v.tensor_scalar(T0[:, c4], stm[:, c4], isC[:, st:st + 1],
                                    None, op0=AL.mult)
                v.tensor_tensor(T0[:, gc], T0[:, gc], LOX[:, gc], op=AL.max)
                v.tensor_tensor(T0[:, gc], T0[:, gc], HIX[:, gc], op=AL.min)

            def clamp(dst, xt, tcol_tile, st):
                cc = st * R
                for j in range(R):
                    nc.gpsimd.tensor_scalar(
                        dst[:, j * K:(j + 1) * K], xt[:, j * K:(j + 1) * K],
                        tcol_tile[:, cc + j:cc + j + 1], EPS,
                        op0=AL.subtract, op1=AL.max)

            def row_sums(wt, dst, st):
                # per-subrow sums via bf16 tensor_scalar accum (4x mode; TTR
                # is broken on this runtime)
                cc = st * R
                for j in range(R):
                    rdt = prd.tile([128, K], BF16, name="rd")
                    v.tensor_scalar(
                        rdt[:, :], wt[:, j * K:(j + 1) * K], 1.0, 0.0,
                        op0=AL.mult, op1=AL.add,
                        accum_out=dst[:, cc + j:cc + j + 1])

            def phaseC(st):
                c4 = slice(st * R, st * R + R)
                STG = DBG_CSTAGE
                sc1 = sC[:, st:st + 1]
                pc1 = pC[:, st:st + 1]
                pm11 = pm1C[:, st:st + 1]
                xt = pxt.tile([128, R * K], F32, name="xt")
                nc.sync.dma_start(sb3(xt[:, :]), x_dram_ap(x_in, st))
                # ---- eval0
                u0 = puu.tile([128, R * K], F32, name="uu")
                clamp(u0, xt, T0, st)
                if STG <= 1:
                    return
                nc.scalar.activation(u0[:, :], u0[:, :], AF.Ln, scale=sc1)
                if STG <= 2:
                    return
                w0 = pwb.tile([128, R * K], BF16, name="wb")
                nc.scalar.activation(w0[:, :], u0[:, :], AF.Exp, scale=pc1)
                if STG <= 3:
                    return
                row_sums(w0, S0, st)
                if STG <= 4:
                    return
                v0 = pwb.tile([128, R * K], BF16, name="wb")
                nc.scalar.activation(v0[:, :], u0[:, :], AF.Exp, scale=pm11)
                row_sums(v0, M1, st)
                if STG <= 5:
                    return
                # ---- Newton: t1 = clip(t0 + h0*S0/(p*s*m1))
                nc.scalar.activation(H0[:, c4], S0[:, c4], AF.Ln)
                v.tensor_tensor(t1a[:, c4], H0[:, c4], S0[:, c4], op=AL.mult)
                v.reciprocal(t1b[:, c4], M1[:, c4])
                v.tensor_tensor(t1a[:, c4], t1a[:, c4], t1b[:, c4],
                                op=AL.mult)
                v.tensor_scalar(t1a[:, c4], t1a[:, c4], ipsC[:, st:st + 1],
                                None, op0=AL.mult)
                v.tensor_tensor(T1[:, c4], T0[:, c4], t1a[:, c4], op=AL.add)
                v.tensor_tensor(T1[:, c4], T1[:, c4], LOX[:, c4], op=AL.max)
                v.tensor_tensor(T1[:, c4], T1[:, c4], HIX[:, c4], op=AL.min)
                if STG <= 6:
                    return
                # ---- eval1
                u1 = puu.tile([128, R * K], F32, name="uu")
                clamp(u1, xt, T1, st)
                nc.scalar.activation(u1[:, :], u1[:, :], AF.Ln, scale=sc1)
                w1 = pwb.tile([128, R * K], BF16, name="wb")
                nc.scalar.activation(w1[:, :], u1[:, :], AF.Exp, scale=pc1)
                row_sums(w1, S1, st)
                if STG <= 7:
                    return
                v1 = pwb.tile([128, R * K], BF16, name="wb")
                nc.scalar.activation(v1[:, :], u1[:, :], AF.Exp, scale=pm11)
                # ---- Newton at t1 -> t3; corr coeff c = p*s*(t1-t3).
                # (A secant here is numerically fragile: on converged rows
                # bf16-rounded w makes S1 == S0 exactly -> 0/0.)
                row_sums(v1, M11, st)
                nc.scalar.activation(t1b[:, c4], S1[:, c4], AF.Ln)  # h1
                v.tensor_tensor(t1a[:, c4], t1b[:, c4], S1[:, c4],
                                op=AL.mult)                          # h1*S1
                v.reciprocal(t1b[:, c4], M11[:, c4])
                v.tensor_tensor(t1a[:, c4], t1a[:, c4], t1b[:, c4],
                                op=AL.mult)
                v.tensor_scalar(t1a[:, c4], t1a[:, c4], ipsC[:, st:st + 1],
                                None, op0=AL.mult)                   # t3-t1
                v.tensor_tensor(t1b[:, c4], T1[:, c4], t1a[:, c4],
                                op=AL.add)                           # t3 raw
                v.tensor_tensor(t1b[:, c4], t1b[:, c4], LOX[:, c4], op=AL.max)
                v.tensor_tensor(t1b[:, c4], t1b[:, c4], HIX[:, c4], op=AL.min)
                v.tensor_tensor(t1a[:, c4], T1[:, c4], t1b[:, c4],
                                op=AL.subtract)                      # t1-t3
                v.tensor_scalar(CCORR[:, c4], t1a[:, c4], psC[:, st:st + 1],
                                None, op0=AL.mult)
                if STG <= 8:
                    return
                # ---- y' = w1 + c*v1 (DVE fused STT, bf16 2x) + S' accum
                yp = pwb.tile([128, R * K], BF16, name="wb")
                cc = st * R
                for j in range(R):
                    v.scalar_tensor_tensor(
                        yp[:, j * K:(j + 1) * K], v1[:, j * K:(j + 1) * K],
                        CCORR[:, cc + j:cc + j + 1],
                        w1[:, j * K:(j + 1) * K],
                        op0=AL.mult, op1=AL.add,
                        accum_out=SP[:, cc + j:cc + j + 1])
                if STG <= 9:
                    return
                v.reciprocal(t1a[:, c4], SP[:, c4])
                yt = puu.tile([128, R * K], F32, name="uu")
                for j in range(R):
                    v.tensor_scalar(yt[:, j * K:(j + 1) * K],
                                    yp[:, j * K:(j + 1) * K],
                                    t1a[:, cc + j:cc + j + 1], None,
                                    op0=AL.mult)
                nc.sync.dma_start(x_dram_ap(y_out, st), sb3(yt[:, :]))

            # ---------- schedule ----------
            ncl = N_ST if DBG_NC < 0 else DBG_NC
            def dump_state():
                if dbg_out is None:
                    return
                for i, t in enumerate((T0, T1, S0, M1, S1, SP, CCORR, MXS,
                                       LOX, HIX, H0, stm)):
                    nc.sync.dma_start(dbg_out[:, i * NC:(i + 1) * NC], t[:, :])
            for st in range(GSTS):
                phaseA(st)
            sketch(0)
            for st in range(GSTS, 2 * GSTS):
                phaseA(st)
            for st in range(0, min(2, ncl)):
                phaseC(st)
            sketch(1)
            for st in range(2 * GSTS, 3 * GSTS):
                phaseA(st)
            for st in range(2, min(5, ncl)):
                phaseC(st)
            sketch(2)
            for st in range(3 * GSTS, 4 * GSTS):
                phaseA(st)
            for st in range(5, min(8, ncl)):
                phaseC(st)
            sketch(3)
            for st in range(8, max(8, ncl)):
                phaseC(st)
            dump_state()

    # Only Ln/Exp are used: keep the one table set holding both so no
    # mid-kernel ACT table reloads are scheduled.
    orig_tables = bacc.get_activation_tables

    def _lnexp_only(arch):
        return {k: (vv if k == "natural_log_exp_and_others" else set())
                for k, vv in orig_tables(arch).items()}

    bacc.get_activation_tables = _lnexp_only
    try:
        nc.finalize()
    finally:
        bacc.get_activation_tables = orig_tables
    return nc


_NC_CACHE = None


def _get_nc():
    global _NC_CACHE
    if _NC_CACHE is None:
        _NC_CACHE = _build()
    return _NC_CACHE


def _make_cst(al, core):
    """Per-core constant table [128, CST_W] fp32."""
    c = np.zeros(CST_W, np.float64)
    for st in range(N_ST):
        h = (core * BLOCKS + st // (Q // ST_ROWS)) % H
        s = al[h] - 1.0
        p = 1.0 / s
        g, gl = st // GSTS, st % GSTS
        # P-tile columns for this st
        base = g * GW + gl * R * NCH
        c[base:base + R * NCH] = p
        c[1024 + st * R:1024 + st * R + R] = 1.0 - (1.0 / K) ** s
        c[1088 + st] = s
        c[1104 + st] = p
        c[1120 + st] = p - 1.0
        c[1136 + st] = 1.0 / s
        c[1152 + st] = 1.0 / (p * s)
        c[1168 + st] = p * s
        c[1184 + st] = (1.0 / K) ** s
    return np.tile(c.astype(np.float32)[None, :], (128, 1))


def kernel(att_scores: np.ndarray, alpha: np.ndarray) -> np.ndarray:
    X = np.ascontiguousarray(np.asarray(att_scores, dtype=np.float32))
    X = X.reshape(B * H, Q, K)
    al = np.asarray(alpha, dtype=np.float64).reshape(H)

    nc = _get_nc()
    in_maps = []
    for c in range(NCORES):
        xc = np.ascontiguousarray(
            X[c * BLOCKS:(c + 1) * BLOCKS].reshape(BLOCKS * Q, K))
        in_maps.append({"x": xc, "cst": _make_cst(al, c)})

    res = run_bass_kernel_spmd(nc, in_maps, core_ids=list(range(NCORES)))
    global LAST_RESULT
    LAST_RESULT = res
    outs = [np.asarray(res.results[c]["y"]) for c in range(NCORES)]
    return np.concatenate(outs, axis=0).reshape(B, H, Q, K).astype(np.float32)


# revision 18
# speedup vs baseline: 1.4218x; 1.0467x over previous
"""Entmax-alpha Bass kernel for Trainium2, 8-core SPMD — sketch+Newton design.

Problem: out = entmax_bisect(att_scores[4,16,1024,1024], alpha[16]) over last
dim; graded metric absmax_rel < 2e-2 (this build reaches ~3e-3).

Algorithm (3 full-data evaluations instead of the reference's 50):
  1. SKETCH: per row, take the 16 chunk-maxes (chunks of 64). Running the
     entmax bisection on this 16-value sketch (6 iters, tiny state tiles)
     nearly exactly solves the PEAKED rows (the ones plain Newton struggles
     with, since S(t) has a kink where elements cross the support threshold).
  2. NEWTON: one full evaluation at t0 computing S0 = sum w and
     m1 = sum (s*u)^(p-1) (an extra Exp over the same Ln output), giving the
     exact local derivative  dlnS/dt = -p*s*m1/S  ->  t1.
  3. OUTPUT + CORRECTION: evaluate w1, S1, v1 = (s*u1)^(p-1), m11 at t1; a
     second Newton step predicts t3; first-order in-place correction
     y ~ (w1 + p*s*(t1-t3)*v1) / (S1 + c*m11), second-order accurate in the
     remaining tau error.  (A secant for t3 is numerically fragile: on
     converged rows bf16-rounded w makes S1 == S0 exactly -> 0/0.)

Per-core device mapping (16 supertiles of [128 part x 4 subrows x 1024]):
  ACT:  6 full passes (Ln, Exp(p), Exp(p-1)) x 2 evals + sketch Ln/Exp
  DVE:  chunk-max reduce, sketch tiles, row-sums via tensor_scalar accum
        (bf16 4x; tensor_tensor_reduce is broken on this runtime), state
        math (pair-batched), correction TS+TT, normalize
  Pool: the per-supertile clamps (tensor_scalar sub+max), SWDGE cast loads
  DMA:  bf16 cast-load (sketch pass), fp32 load (eval pass), fp32 store
Work is emitted stage-interleaved (round-robin over supertile pairs) so each
engine's in-order queue always has independent work; per-head constants come
from the per-core cst input (single SPMD NEFF).
"""

import numpy as np

import concourse.bacc as bacc
import concourse.mybir as mybir
from concourse.tile import TileContext
from concourse.bass_utils import run_bass_kernel_spmd

B, H, Q, K = 4, 16, 1024, 1024
NCORES = 8
BLOCKS = (B * H) // NCORES      # head-blocks per core (8)
R = 4                           # q-subrows per partition per supertile
ST_ROWS = 128 * R               # rows per supertile (512)
N_ST = BLOCKS * Q // ST_ROWS    # supertiles per core (16)
NC = N_ST * R                   # state columns (64)
NCH = 16                        # sketch chunks per row
CHW = K // NCH                  # chunk width (64)
import os as _os
SK_ITERS = int(_os.environ.get("SKITERS", "6"))
SK_GROUPS = 4                   # sketch groups
GSTS = N_ST // SK_GROUPS        # sts per group (4)
GW = GSTS * R * NCH             # sketch tile width per group (256)
GS = GSTS * R                   # state cols per group (16)
EPS = 1e-30

AL = mybir.AluOpType
AF = mybir.ActivationFunctionType
F32 = mybir.dt.float32
BF16 = mybir.dt.bfloat16
U8 = mybir.dt.uint8

# cst layout (fp32, replicated across 128 partitions):
#   [0:1024)                 P-tiles: p per sketch column, 4 groups x 256
#   [1024:1088)              isC  per (st,j): 1/s
#   [1088:1152)              ipsC per (st,j): 1/(p*s)
#   [1152:1216)              psC  per (st,j): p*s
#   [1216:1280)              ntcC per (st,j): (1/K)^s
#   [1280:1280+6*64)         DMI[i] per (st,j): (1-(1/K)^s) * 2^-(i+1)
#   [1664:1680)              sC   per st: s    (activation scale columns)
#   [1680:1696)              pC   per st: p
#   [1696:1712)              pm1C per st: p-1
CST_W = 1712

LAST_RESULT = None


def _build():
    nc = bacc.Bacc(None, target_bir_lowering=False)
    x_in = nc.declare_dram_parameter("x", [BLOCKS * Q, K], F32, isOutput=False)
    cst_in = nc.declare_dram_parameter("cst", [128, CST_W], F32, isOutput=False)
    y_out = nc.declare_dram_parameter("y", [BLOCKS * Q, K], F32, isOutput=True)

    def x_dram_ap(handle, st):
        r0 = st * ST_ROWS
        return handle[r0:r0 + ST_ROWS, :].rearrange("(j p) k -> p j k", p=128)

    def sb3(tile_ap):
        return tile_ap.rearrange("p (j k) -> p j k", k=K)

    with TileContext(nc) as tc:
        with tc.tile_pool(name="state", bufs=1) as stp, \
             tc.tile_pool(name="xa", bufs=2) as pxa, \
             tc.tile_pool(name="xt", bufs=3) as pxt, \
             tc.tile_pool(name="uu", bufs=4) as puu, \
             tc.tile_pool(name="wb", bufs=7) as pwb, \
             tc.tile_pool(name="rd", bufs=2) as prd, \
             tc.tile_pool(name="skw", bufs=2) as psk:
            v = nc.vector

            cst = stp.tile([128, CST_W], F32)
            nc.sync.dma_start(cst[:, :], cst_in[:, :])
            PT = cst[:, 0:1024]
            isC = cst[:, 1024:1088]
            ipsC = cst[:, 1088:1152]
            psC = cst[:, 1152:1216]
            ntcC = cst[:, 1216:1280]
            DMI = [cst[:, 1280 + i * NC:1280 + (i + 1) * NC] for i in range(6)]
            sC = cst[:, 1664:1680]
            pC = cst[:, 1680:1696]
            pm1C = cst[:, 1696:1712]

            CM = stp.tile([128, NC * NCH], BF16)     # s-scaled chunk maxes
            MXS = stp.tile([128, NC], F32)          # row max (s-domain)
            T0 = stp.tile([128, NC], F32)           # x-domain
            T1 = stp.tile([128, NC], F32)
            LOX = stp.tile([128, NC], F32)
            HIX = stp.tile([128, NC], F32)
            H0 = stp.tile([128, NC], F32)
            S0 = stp.tile([128, NC], F32)
            M1 = stp.tile([128, NC], F32)
            S1 = stp.tile([128, NC], F32)
            M11 = stp.tile([128, NC], F32)
            SP = stp.tile([128, NC], F32)
            CCORR = stp.tile([128, NC], F32)
            t1a = stp.tile([128, NC], F32)
            t1b = stp.tile([128, NC], F32)
            t1c = stp.tile([128, NC], F32)
            slo = stp.tile([128, NC], F32)
            stm = stp.tile([128, NC], F32)
            smask = stp.tile([128, NC], U8)
            ssum = stp.tile([128, NC], F32)

            def row_sums(wt, dst, st, j):
                # per-subrow sum via bf16 tensor_scalar accum (4x mode)
                rdt = prd.tile([128, K], BF16, name="rd")
                v.tensor_scalar(
                    rdt[:, :], wt[:, j * K:(j + 1) * K], 1.0, 0.0,
                    op0=AL.mult, op1=AL.add,
                    accum_out=dst[:, st * R + j:st * R + j + 1])

            def clamp(dst, xt, tcol, st):
                cc = st * R
                for j in range(R):
                    nc.gpsimd.tensor_scalar(
                        dst[:, j * K:(j + 1) * K], xt[:, j * K:(j + 1) * K],
                        tcol[:, cc + j:cc + j + 1], EPS,
                        op0=AL.subtract, op1=AL.max)

            # ---------------- work-unit generators ----------------
            def genA(st):
                xa = pxa.tile([128, R * K], BF16, name="xa")
                nc.gpsimd.dma_start(sb3(xa[:, :]), x_dram_ap(x_in, st))
                yield
                cmsl = CM[:, st * R * NCH:(st + 1) * R * NCH]
                v.tensor_reduce(
                    cmsl.rearrange("p (j c) -> p j c", c=NCH),
                    xa[:, :].rearrange("p (j c k) -> p j c k", c=NCH, k=CHW),
                    axis=mybir.AxisListType.X, op=AL.max)
                v.tensor_scalar(cmsl, cmsl, sC[:, st:st + 1], None,
                                op0=AL.mult)
                c4 = slice(st * R, st * R + R)
                v.tensor_reduce(MXS[:, c4],
                                cmsl.rearrange("p (j c) -> p j c", c=NCH),
                                axis=mybir.AxisListType.X, op=AL.max)
                yield

            def genSketch(g):
                gc = slice(g * GS, (g + 1) * GS)
                gw = slice(g * GS * NCH, (g + 1) * GS * NCH)
                cm = CM[:, gw]
                # brackets (x units) for the whole group
                v.tensor_scalar(t1a[:, gc], MXS[:, gc], 1.0, None,
                                op0=AL.subtract)
                v.tensor_tensor(LOX[:, gc], t1a[:, gc], isC[:, gc],
                                op=AL.mult)
                v.tensor_tensor(t1b[:, gc], MXS[:, gc], ntcC[:, gc],
                                op=AL.subtract)
                v.tensor_tensor(HIX[:, gc], t1b[:, gc], isC[:, gc],
                                op=AL.mult)
                v.tensor_scalar(slo[:, gc], MXS[:, gc], 1.0, None,
                                op0=AL.subtract)
                yield
                for it in range(SK_ITERS):
                    v.tensor_tensor(stm[:, gc], slo[:, gc], DMI[it][:, gc],
                                    op=AL.add)
                    wt = psk.tile([128, GW], F32, name="skw")
                    v.tensor_tensor(
                        wt[:, :].rearrange("p (s c) -> p s c", c=NCH),
                        cm.rearrange("p (s c) -> p s c", c=NCH),
                        stm[:, gc].rearrange("p (s o) -> p s o", o=1)
                        .broadcast_to((128, GS, NCH)),
                        op=AL.subtract)
                    v.tensor_scalar(wt[:, :], wt[:, :], EPS, None, op0=AL.max)
                    nc.scalar.activation(wt[:, :], wt[:, :], AF.Ln)
                    v.tensor_tensor(wt[:, :], wt[:, :], PT[:, gw], op=AL.mult)
                    nc.scalar.activation(wt[:, :], wt[:, :], AF.Exp)
                    v.tensor_reduce(ssum[:, gc],
                                    wt[:, :].rearrange("p (s c) -> p s c",
                                                       c=NCH),
                                    axis=mybir.AxisListType.X, op=AL.add)
                    v.tensor_scalar(smask[:, gc], ssum[:, gc], 1.0, None,
                                    op0=AL.is_ge)
                    v.copy_predicated(slo[:, gc], smask[:, gc], stm[:, gc])
                    yield
                v.tensor_tensor(stm[:, gc], slo[:, gc],
                                DMI[SK_ITERS - 1][:, gc], op=AL.add)
                v.tensor_tensor(T0[:, gc], stm[:, gc], isC[:, gc], op=AL.mult)
                v.tensor_tensor(T0[:, gc], T0[:, gc], LOX[:, gc], op=AL.max)
                v.tensor_tensor(T0[:, gc], T0[:, gc], HIX[:, gc], op=AL.min)
                yield

            def genC(pr):
                """Pipeline for the supertile pair (2*pr, 2*pr+1)."""
                sts = (2 * pr, 2 * pr + 1)
                c8 = slice(sts[0] * R, sts[0] * R + 2 * R)
                xts, u0s, w0s, v1s, w1s = {}, {}, {}, {}, {}
                for st in sts:
                    xt = pxt.tile([128, R * K], F32, name="xt")
                    nc.sync.dma_start(sb3(xt[:, :]), x_dram_ap(x_in, st))
                    xts[st] = xt
                yield
                for st in sts:
                    u0 = puu.tile([128, R * K], F32, name="uu")
                    clamp(u0, xts[st], T0, st)
                    u0s[st] = u0
                    yield
                xts = {}
                for st in sts:
                    nc.scalar.activation(u0s[st][:, :], u0s[st][:, :], AF.Ln,
                                         scale=sC[:, st:st + 1])
                    yield
                for st in sts:
                    w0 = pwb.tile([128, R * K], BF16, name="wb")
                    nc.scalar.activation(w0[:, :], u0s[st][:, :], AF.Exp,
                                         scale=pC[:, st:st + 1])
                    for j in range(R):
                        row_sums(w0, S0, st, j)
                    yield
                for st in sts:
                    v0 = pwb.tile([128, R * K], BF16, name="wb")
                    nc.scalar.activation(v0[:, :], u0s[st][:, :], AF.Exp,
                                         scale=pm1C[:, st:st + 1])
                    for j in range(R):
                        row_sums(v0, M1, st, j)
                    yield
                # Newton (pair-batched): t1 = clip(t0 + h0*S0/(p*s*m1))
                nc.scalar.activation(H0[:, c8], S0[:, c8], AF.Ln)
                v.tensor_tensor(t1a[:, c8], H0[:, c8], S0[:, c8], op=AL.mult)
                v.reciprocal(t1b[:, c8], M1[:, c8])
                v.tensor_tensor(t1a[:, c8], t1a[:, c8], t1b[:, c8],
                                op=AL.mult)
                v.tensor_tensor(t1a[:, c8], t1a[:, c8], ipsC[:, c8],
                                op=AL.mult)
                v.tensor_tensor(T1[:, c8], T0[:, c8], t1a[:, c8], op=AL.add)
                v.tensor_tensor(T1[:, c8], T1[:, c8], LOX[:, c8], op=AL.max)
                v.tensor_tensor(T1[:, c8], T1[:, c8], HIX[:, c8], op=AL.min)
                yield
                xt1s = {}
                for st in sts:
                    xt = pxt.tile([128, R * K], F32, name="xt")
                    nc.sync.dma_start(sb3(xt[:, :]), x_dram_ap(x_in, st))
                    xt1s[st] = xt
                yield
                u1s = {}
                for st in sts:
                    u1 = puu.tile([128, R * K], F32, name="uu")
                    clamp(u1, xt1s[st], T1, st)
                    u1s[st] = u1
                    yield
                xt1s = {}
                for st in sts:
                    nc.scalar.activation(u1s[st][:, :], u1s[st][:, :], AF.Ln,
                                         scale=sC[:, st:st + 1])
                    yield
                for st in sts:
                    w1 = pwb.tile([128, R * K], BF16, name="wb")
                    nc.scalar.activation(w1[:, :], u1s[st][:, :], AF.Exp,
                                         scale=pC[:, st:st + 1])
                    w1s[st] = w1
                    for j in range(R):
                        row_sums(w1, S1, st, j)
                    yield
                for st in sts:
                    v1 = pwb.tile([128, R * K], BF16, name="wb")
                    nc.scalar.activation(v1[:, :], u1s[st][:, :], AF.Exp,
                                         scale=pm1C[:, st:st + 1])
                    v1s[st] = v1
                    for j in range(R):
                        row_sums(v1, M11, st, j)
                    yield
                # Newton at t1 -> t3; c = p*s*(t1-t3); S' = S1 + c*m11
                nc.scalar.activation(t1b[:, c8], S1[:, c8], AF.Ln)
                v.tensor_tensor(t1a[:, c8], t1b[:, c8], S1[:, c8],
                                op=AL.mult)
                v.reciprocal(t1c[:, c8], M11[:, c8])
                v.tensor_tensor(t1a[:, c8], t1a[:, c8], t1c[:, c8],
                                op=AL.mult)
                v.tensor_tensor(t1a[:, c8], t1a[:, c8], ipsC[:, c8],
                                op=AL.mult)                          # t3-t1
                v.tensor_tensor(t1b[:, c8], T1[:, c8], t1a[:, c8], op=AL.add)
                v.tensor_tensor(t1b[:, c8], t1b[:, c8], LOX[:, c8], op=AL.max)
                v.tensor_tensor(t1b[:, c8], t1b[:, c8], HIX[:, c8], op=AL.min)
                v.tensor_tensor(t1a[:, c8], T1[:, c8], t1b[:, c8],
                                op=AL.subtract)                      # t1-t3
                v.tensor_tensor(CCORR[:, c8], t1a[:, c8], psC[:, c8],
                                op=AL.mult)
                v.tensor_tensor(t1a[:, c8], CCORR[:, c8], M11[:, c8],
                                op=AL.mult)
                v.tensor_tensor(SP[:, c8], S1[:, c8], t1a[:, c8], op=AL.add)
                v.reciprocal(t1c[:, c8], SP[:, c8])
                yield
                for st in sts:
                    cc = st * R
                    gt = pwb.tile([128, R * K], BF16, name="wb")
                    for j in range(R):
                        v.tensor_scalar(gt[:, j * K:(j + 1) * K],
                                        v1s[st][:, j * K:(j + 1) * K],
                                        CCORR[:, cc + j:cc + j + 1], None,
                                        op0=AL.mult)
                    yp = pwb.tile([128, R * K], BF16, name="wb")
                    v.tensor_tensor(yp[:, :], w1s[st][:, :], gt[:, :],
                                    op=AL.add)
                    yield
                    yt = puu.tile([128, R * K], F32, name="uu")
                    for j in range(R):
                        v.tensor_scalar(yt[:, j * K:(j + 1) * K],
                                        yp[:, j * K:(j + 1) * K],
                                        t1c[:, cc + j:cc + j + 1], None,
                                        op0=AL.mult)
                    nc.sync.dma_start(x_dram_ap(y_out, st), sb3(yt[:, :]))
                    yield

            # ---------------- round-robin scheduler ----------------
            def drain(gens, n=1):
                for _ in range(n):
                    for gg in list(gens):
                        try:
                            next(gg)
                        except StopIteration:
                            gens.remove(gg)

            a_gens = {st: genA(st) for st in range(N_ST)}
            sk = {g: genSketch(g) for g in range(SK_GROUPS)}
            c_gens = {pr: genC(pr) for pr in range(N_ST // 2)}

            gens = [a_gens.pop(st) for st in range(GSTS)]
            drain(gens, 3)
            gens.append(sk.pop(0))
            rest = list(range(GSTS, N_ST))
            while rest:
                st = rest.pop(0)
                gens.append(a_gens.pop(st))
                drain(gens, 1)
                if st == 2 * GSTS - 1:
                    gens.append(sk.pop(1))
                if st == 3 * GSTS - 1:
                    gens.append(sk.pop(2))
                if st == 4 * GSTS - 1:
                    gens.append(sk.pop(3))
            # phase C pairs: at most 2 in flight, the second staggered by
            # half a pipeline so peak tile footprint stays bounded
            live = list(gens)
            nprs = N_ST // 2
            pending = list(range(nprs))
            cs = []
            prog = {}
            STAG = 12
            while pending or cs or live:
                if pending and (not cs or
                                (len(cs) == 1 and prog[id(cs[0])] >= STAG)):
                    g = c_gens.pop(pending.pop(0))
                    cs.append(g)
                    prog[id(g)] = 0
                drain(live, 1)
                for g in list(cs):
                    try:
                        next(g)
                        prog[id(g)] += 1
                    except StopIteration:
                        cs.remove(g)

    orig_tables = bacc.get_activation_tables

    def _lnexp_only(arch):
        return {k: (vv if k == "natural_log_exp_and_others" else set())
                for k, vv in orig_tables(arch).items()}

    bacc.get_activation_tables = _lnexp_only
    try:
        nc.finalize()
    finally:
        bacc.get_activation_tables = orig_tables
    return nc


_NC_CACHE = None


def _get_nc():
    global _NC_CACHE
    if _NC_CACHE is None:
        _NC_CACHE = _build()
    return _NC_CACHE


def _make_cst(al, core):
    """Per-core constant table [128, CST_W] fp32."""
    c = np.zeros(CST_W, np.float64)
    for st in range(N_ST):
        h = (core * BLOCKS + st // (Q // ST_ROWS)) % H
        s = al[h] - 1.0
        p = 1.0 / s
        g, gl = st // GSTS, st % GSTS
        base = g * GW + gl * R * NCH
        c[base:base + R * NCH] = p
        c[1024 + st * R:1024 + st * R + R] = 1.0 / s
        c[1088 + st * R:1088 + st * R + R] = 1.0 / (p * s)
        c[1152 + st * R:1152 + st * R + R] = p * s
        c[1216 + st * R:1216 + st * R + R] = (1.0 / K) ** s
        dm0 = 1.0 - (1.0 / K) ** s
        for i in range(6):
            c[1280 + i * NC + st * R:1280 + i * NC + st * R + R] = \
                dm0 * (0.5 ** (i + 1))
        c[1664 + st] = s
        c[1680 + st] = p
        c[1696 + st] = p - 1.0
    return np.tile(c.astype(np.float32)[None, :], (128, 1))


def kernel(att_scores: np.ndarray, alpha: np.ndarray) -> np.ndarray:
    X = np.ascontiguousarray(np.asarray(att_scores, dtype=np.float32))
    X = X.reshape(B * H, Q, K)
    al = np.asarray(alpha, dtype=np.float64).reshape(H)

    nc = _get_nc()
    in_maps = []
    for c in range(NCORES):
        xc = np.ascontiguousarray(
            X[c * BLOCKS:(c + 1) * BLOCKS].reshape(BLOCKS * Q, K))
        in_maps.append({"x": xc, "cst": _make_cst(al, c)})

    res = run_bass_kernel_spmd(nc, in_maps, core_ids=list(range(NCORES)))
    global LAST_RESULT
    LAST_RESULT = res
    outs = [np.asarray(res.results[c]["y"]) for c in range(NCORES)]
    return np.concatenate(outs, axis=0).reshape(B, H, Q, K).astype(np.float32)


# revision 19
# speedup vs baseline: 1.4218x; 1.0001x over previous
"""Entmax-alpha Bass kernel for Trainium2, 8-core SPMD — sketch+Newton design.

Problem: out = entmax_bisect(att_scores[4,16,1024,1024], alpha[16]) over last
dim; graded metric absmax_rel < 2e-2 (this build reaches ~3e-3).

Algorithm (3 full-data evaluations instead of the reference's 50):
  1. SKETCH: per row, take the 16 chunk-maxes (chunks of 64). Running the
     entmax bisection on this 16-value sketch (6 iters, tiny state tiles)
     nearly exactly solves the PEAKED rows (the ones plain Newton struggles
     with, since S(t) has a kink where elements cross the support threshold).
  2. NEWTON: one full evaluation at t0 computing S0 = sum w and
     m1 = sum (s*u)^(p-1) (an extra Exp over the same Ln output), giving the
     exact local derivative  dlnS/dt = -p*s*m1/S  ->  t1.
  3. OUTPUT + CORRECTION: evaluate w1, S1, v1 = (s*u1)^(p-1), m11 at t1; a
     second Newton step predicts t3; first-order in-place correction
     y ~ (w1 + p*s*(t1-t3)*v1) / (S1 + c*m11), second-order accurate in the
     remaining tau error.  (A secant for t3 is numerically fragile: on
     converged rows bf16-rounded w makes S1 == S0 exactly -> 0/0.)

Per-core device mapping (16 supertiles of [128 part x 4 subrows x 1024]):
  ACT:  6 full passes (Ln, Exp(p), Exp(p-1)) x 2 evals + sketch Ln/Exp
  DVE:  chunk-max reduce, sketch tiles, row-sums via tensor_scalar accum
        (bf16 4x; tensor_tensor_reduce is broken on this runtime), state
        math (pair-batched), correction TS+TT, normalize
  Pool: the per-supertile clamps (tensor_scalar sub+max), SWDGE cast loads
  DMA:  bf16 cast-load (sketch pass), fp32 load (eval pass), fp32 store
Work is emitted stage-interleaved (round-robin over supertile pairs) so each
engine's in-order queue always has independent work; per-head constants come
from the per-core cst input (single SPMD NEFF).
"""

import numpy as np

import concourse.bacc as bacc
import concourse.mybir as mybir
from concourse.tile import TileContext
from concourse.bass_utils import run_bass_kernel_spmd

B, H, Q, K = 4, 16, 1024, 1024
NCORES = 8
BLOCKS = (B * H) // NCORES      # head-blocks per core (8)
R = 4                           # q-subrows per partition per supertile
ST_ROWS = 128 * R               # rows per supertile (512)
N_ST = BLOCKS * Q // ST_ROWS    # supertiles per core (16)
NC = N_ST * R                   # state columns (64)
NCH = 16                        # sketch chunks per row
CHW = K // NCH                  # chunk width (64)
import os as _os
SK_ITERS = int(_os.environ.get("SKITERS", "6"))
SK_GROUPS = 4                   # sketch groups
GSTS = N_ST // SK_GROUPS        # sts per group (4)
GW = GSTS * R * NCH             # sketch tile width per group (256)
GS = GSTS * R                   # state cols per group (16)
EPS = 1e-30

AL = mybir.AluOpType
AF = mybir.ActivationFunctionType
F32 = mybir.dt.float32
BF16 = mybir.dt.bfloat16
U8 = mybir.dt.uint8

# cst layout (fp32, replicated across 128 partitions):
#   [0:1024)                 P-tiles: p per sketch column, 4 groups x 256
#   [1024:1088)              isC  per (st,j): 1/s
#   [1088:1152)              ipsC per (st,j): 1/(p*s)
#   [1152:1216)              psC  per (st,j): p*s
#   [1216:1280)              ntcC per (st,j): (1/K)^s
#   [1280:1280+6*64)         DMI[i] per (st,j): (1-(1/K)^s) * 2^-(i+1)
#   [1664:1680)              sC   per st: s    (activation scale columns)
#   [1680:1696)              pC   per st: p
#   [1696:1712)              pm1C per st: p-1
CST_W = 1712

LAST_RESULT = None


def _build():
    nc = bacc.Bacc(None, target_bir_lowering=False)
    x_in = nc.declare_dram_parameter("x", [BLOCKS * Q, K], F32, isOutput=False)
    cst_in = nc.declare_dram_parameter("cst", [128, CST_W], F32, isOutput=False)
    y_out = nc.declare_dram_parameter("y", [BLOCKS * Q, K], F32, isOutput=True)

    def x_dram_ap(handle, st):
        r0 = st * ST_ROWS
        return handle[r0:r0 + ST_ROWS, :].rearrange("(j p) k -> p j k", p=128)

    def sb3(tile_ap):
        return tile_ap.rearrange("p (j k) -> p j k", k=K)

    with TileContext(nc) as tc:
        with tc.tile_pool(name="state", bufs=1) as stp, \
             tc.tile_pool(name="xa", bufs=2) as pxa, \
             tc.tile_pool(name="xt", bufs=3) as pxt, \
             tc.tile_pool(name="uu", bufs=4) as puu, \
             tc.tile_pool(name="wb", bufs=7) as pwb, \
             tc.tile_pool(name="rd", bufs=2) as prd, \
             tc.tile_pool(name="skw", bufs=2) as psk:
            v = nc.vector

            cst = stp.tile([128, CST_W], F32)
            nc.sync.dma_start(cst[:, :], cst_in[:, :])
            PT = cst[:, 0:1024]
            isC = cst[:, 1024:1088]
            ipsC = cst[:, 1088:1152]
            psC = cst[:, 1152:1216]
            ntcC = cst[:, 1216:1280]
            DMI = [cst[:, 1280 + i * NC:1280 + (i + 1) * NC] for i in range(6)]
            sC = cst[:, 1664:1680]
            pC = cst[:, 1680:1696]
            pm1C = cst[:, 1696:1712]

            CM = stp.tile([128, NC * NCH], BF16)     # s-scaled chunk maxes
            MXS = stp.tile([128, NC], F32)          # row max (s-domain)
            T0 = stp.tile([128, NC], F32)           # x-domain
            T1 = stp.tile([128, NC], F32)
            LOX = stp.tile([128, NC], F32)
            HIX = stp.tile([128, NC], F32)
            H0 = stp.tile([128, NC], F32)
            S0 = stp.tile([128, NC], F32)
            M1 = stp.tile([128, NC], F32)
            S1 = stp.tile([128, NC], F32)
            M11 = stp.tile([128, NC], F32)
            SP = stp.tile([128, NC], F32)
            CCORR = stp.tile([128, NC], F32)
            t1a = stp.tile([128, NC], F32)
            t1b = stp.tile([128, NC], F32)
            t1c = stp.tile([128, NC], F32)
            slo = stp.tile([128, NC], F32)
            stm = stp.tile([128, NC], F32)
            smask = stp.tile([128, NC], U8)
            ssum = stp.tile([128, NC], F32)

            def row_sums(wt, dst, st, j):
                # per-subrow sum via bf16 tensor_scalar accum (4x mode)
                rdt = prd.tile([128, K], BF16, name="rd")
                v.tensor_scalar(
                    rdt[:, :], wt[:, j * K:(j + 1) * K], 1.0, 0.0,
                    op0=AL.mult, op1=AL.add,
                    accum_out=dst[:, st * R + j:st * R + j + 1])

            def clamp(dst, xt, tcol, st):
                cc = st * R
                for j in range(R):
                    nc.gpsimd.tensor_scalar(
                        dst[:, j * K:(j + 1) * K], xt[:, j * K:(j + 1) * K],
                        tcol[:, cc + j:cc + j + 1], EPS,
                        op0=AL.subtract, op1=AL.max)

            # ---------------- work-unit generators ----------------
            xa_tiles = {}

            def loadA(st):
                xa = pxa.tile([128, R * K], BF16, name="xa")
                nc.gpsimd.dma_start(sb3(xa[:, :]), x_dram_ap(x_in, st))
                xa_tiles[st] = xa

            def genA(st):
                xa = xa_tiles.pop(st)
                cmsl = CM[:, st * R * NCH:(st + 1) * R * NCH]
                v.tensor_reduce(
                    cmsl.rearrange("p (j c) -> p j c", c=NCH),
                    xa[:, :].rearrange("p (j c k) -> p j c k", c=NCH, k=CHW),
                    axis=mybir.AxisListType.X, op=AL.max)
                v.tensor_scalar(cmsl, cmsl, sC[:, st:st + 1], None,
                                op0=AL.mult)
                c4 = slice(st * R, st * R + R)
                v.tensor_reduce(MXS[:, c4],
                                cmsl.rearrange("p (j c) -> p j c", c=NCH),
                                axis=mybir.AxisListType.X, op=AL.max)
                yield

            def genSketch(g):
                gc = slice(g * GS, (g + 1) * GS)
                gw = slice(g * GS * NCH, (g + 1) * GS * NCH)
                cm = CM[:, gw]
                # brackets (x units) for the whole group
                v.tensor_scalar(t1a[:, gc], MXS[:, gc], 1.0, None,
                                op0=AL.subtract)
                v.tensor_tensor(LOX[:, gc], t1a[:, gc], isC[:, gc],
                                op=AL.mult)
                v.tensor_tensor(t1b[:, gc], MXS[:, gc], ntcC[:, gc],
                                op=AL.subtract)
                v.tensor_tensor(HIX[:, gc], t1b[:, gc], isC[:, gc],
                                op=AL.mult)
                v.tensor_scalar(slo[:, gc], MXS[:, gc], 1.0, None,
                                op0=AL.subtract)
                yield
                for it in range(SK_ITERS):
                    v.tensor_tensor(stm[:, gc], slo[:, gc], DMI[it][:, gc],
                                    op=AL.add)
                    wt = psk.tile([128, GW], F32, name="skw")
                    v.tensor_tensor(
                        wt[:, :].rearrange("p (s c) -> p s c", c=NCH),
                        cm.rearrange("p (s c) -> p s c", c=NCH),
                        stm[:, gc].rearrange("p (s o) -> p s o", o=1)
                        .broadcast_to((128, GS, NCH)),
                        op=AL.subtract)
                    v.tensor_scalar(wt[:, :], wt[:, :], EPS, None, op0=AL.max)
                    nc.scalar.activation(wt[:, :], wt[:, :], AF.Ln)
                    v.tensor_tensor(wt[:, :], wt[:, :], PT[:, gw], op=AL.mult)
                    nc.scalar.activation(wt[:, :], wt[:, :], AF.Exp)
                    v.tensor_reduce(ssum[:, gc],
                                    wt[:, :].rearrange("p (s c) -> p s c",
                                                       c=NCH),
                                    axis=mybir.AxisListType.X, op=AL.add)
                    v.tensor_scalar(smask[:, gc], ssum[:, gc], 1.0, None,
                                    op0=AL.is_ge)
                    v.copy_predicated(slo[:, gc], smask[:, gc], stm[:, gc])
                    yield
                v.tensor_tensor(stm[:, gc], slo[:, gc],
                                DMI[SK_ITERS - 1][:, gc], op=AL.add)
                v.tensor_tensor(T0[:, gc], stm[:, gc], isC[:, gc], op=AL.mult)
                v.tensor_tensor(T0[:, gc], T0[:, gc], LOX[:, gc], op=AL.max)
                v.tensor_tensor(T0[:, gc], T0[:, gc], HIX[:, gc], op=AL.min)
                yield

            def genC(pr):
                """Pipeline for the supertile pair (2*pr, 2*pr+1)."""
                sts = (2 * pr, 2 * pr + 1)
                c8 = slice(sts[0] * R, sts[0] * R + 2 * R)
                xts, u0s, w0s, v1s, w1s = {}, {}, {}, {}, {}
                for st in sts:
                    xt = pxt.tile([128, R * K], F32, name="xt")
                    nc.sync.dma_start(sb3(xt[:, :]), x_dram_ap(x_in, st))
                    xts[st] = xt
                yield
                for st in sts:
                    u0 = puu.tile([128, R * K], F32, name="uu")
                    clamp(u0, xts[st], T0, st)
                    u0s[st] = u0
                    yield
                xts = {}
                for st in sts:
                    nc.scalar.activation(u0s[st][:, :], u0s[st][:, :], AF.Ln,
                                         scale=sC[:, st:st + 1])
                    yield
                for st in sts:
                    w0 = pwb.tile([128, R * K], BF16, name="wb")
                    nc.scalar.activation(w0[:, :], u0s[st][:, :], AF.Exp,
                                         scale=pC[:, st:st + 1])
                    for j in range(R):
                        row_sums(w0, S0, st, j)
                    yield
                xt1s = {}
                for st in sts:
                    v0 = pwb.tile([128, R * K], BF16, name="wb")
                    nc.scalar.activation(v0[:, :], u0s[st][:, :], AF.Exp,
                                         scale=pm1C[:, st:st + 1])
                    for j in range(R):
                        row_sums(v0, M1, st, j)
                    # prefetch the eval1 copy of x (not t1-dependent)
                    xt = pxt.tile([128, R * K], F32, name="xt")
                    nc.sync.dma_start(sb3(xt[:, :]), x_dram_ap(x_in, st))
                    xt1s[st] = xt
                    yield
                # Newton (pair-batched): t1 = clip(t0 + h0*S0/(p*s*m1))
                nc.scalar.activation(H0[:, c8], S0[:, c8], AF.Ln)
                v.tensor_tensor(t1a[:, c8], H0[:, c8], S0[:, c8], op=AL.mult)
                v.reciprocal(t1b[:, c8], M1[:, c8])
                v.tensor_tensor(t1a[:, c8], t1a[:, c8], t1b[:, c8],
                                op=AL.mult)
                v.tensor_tensor(t1a[:, c8], t1a[:, c8], ipsC[:, c8],
                                op=AL.mult)
                v.tensor_tensor(T1[:, c8], T0[:, c8], t1a[:, c8], op=AL.add)
                v.tensor_tensor(T1[:, c8], T1[:, c8], LOX[:, c8], op=AL.max)
                v.tensor_tensor(T1[:, c8], T1[:, c8], HIX[:, c8], op=AL.min)
                yield
                u1s = {}
                for st in sts:
                    u1 = puu.tile([128, R * K], F32, name="uu")
                    clamp(u1, xt1s[st], T1, st)
                    u1s[st] = u1
                    yield
                xt1s = {}
                for st in sts:
                    nc.scalar.activation(u1s[st][:, :], u1s[st][:, :], AF.Ln,
                                         scale=sC[:, st:st + 1])
                    yield
                for st in sts:
                    w1 = pwb.tile([128, R * K], BF16, name="wb")
                    nc.scalar.activation(w1[:, :], u1s[st][:, :], AF.Exp,
                                         scale=pC[:, st:st + 1])
                    w1s[st] = w1
                    for j in range(R):
                        row_sums(w1, S1, st, j)
                    yield
                for st in sts:
                    v1 = pwb.tile([128, R * K], BF16, name="wb")
                    nc.scalar.activation(v1[:, :], u1s[st][:, :], AF.Exp,
                                         scale=pm1C[:, st:st + 1])
                    v1s[st] = v1
                    for j in range(R):
                        row_sums(v1, M11, st, j)
                    yield
                # Newton at t1 -> t3; c = p*s*(t1-t3); S' = S1 + c*m11
                nc.scalar.activation(t1b[:, c8], S1[:, c8], AF.Ln)
                v.tensor_tensor(t1a[:, c8], t1b[:, c8], S1[:, c8],
                                op=AL.mult)
                v.reciprocal(t1c[:, c8], M11[:, c8])
                v.tensor_tensor(t1a[:, c8], t1a[:, c8], t1c[:, c8],
                                op=AL.mult)
                v.tensor_tensor(t1a[:, c8], t1a[:, c8], ipsC[:, c8],
                                op=AL.mult)                          # t3-t1
                v.tensor_tensor(t1b[:, c8], T1[:, c8], t1a[:, c8], op=AL.add)
                v.tensor_tensor(t1b[:, c8], t1b[:, c8], LOX[:, c8], op=AL.max)
                v.tensor_tensor(t1b[:, c8], t1b[:, c8], HIX[:, c8], op=AL.min)
                v.tensor_tensor(t1a[:, c8], T1[:, c8], t1b[:, c8],
                                op=AL.subtract)                      # t1-t3
                v.tensor_tensor(CCORR[:, c8], t1a[:, c8], psC[:, c8],
                                op=AL.mult)
                v.tensor_tensor(t1a[:, c8], CCORR[:, c8], M11[:, c8],
                                op=AL.mult)
                v.tensor_tensor(SP[:, c8], S1[:, c8], t1a[:, c8], op=AL.add)
                v.reciprocal(t1c[:, c8], SP[:, c8])
                yield
                for st in sts:
                    cc = st * R
                    gt = pwb.tile([128, R * K], BF16, name="wb")
                    for j in range(R):
                        v.tensor_scalar(gt[:, j * K:(j + 1) * K],
                                        v1s[st][:, j * K:(j + 1) * K],
                                        CCORR[:, cc + j:cc + j + 1], None,
                                        op0=AL.mult)
                    yp = pwb.tile([128, R * K], BF16, name="wb")
                    v.tensor_tensor(yp[:, :], w1s[st][:, :], gt[:, :],
                                    op=AL.add)
                    yield
                    yt = puu.tile([128, R * K], F32, name="uu")
                    for j in range(R):
                        v.tensor_scalar(yt[:, j * K:(j + 1) * K],
                                        yp[:, j * K:(j + 1) * K],
                                        t1c[:, cc + j:cc + j + 1], None,
                                        op0=AL.mult)
                    nc.sync.dma_start(x_dram_ap(y_out, st), sb3(yt[:, :]))
                    yield

            # ---------------- round-robin scheduler ----------------
            def drain(gens, n=1):
                for _ in range(n):
                    for gg in list(gens):
                        try:
                            next(gg)
                        except StopIteration:
                            gens.remove(gg)

            sk = {g: genSketch(g) for g in range(SK_GROUPS)}
            c_gens = {pr: genC(pr) for pr in range(N_ST // 2)}

            # all of group 0+1 bf16 loads queued first; cm+sketch(0) solo so
            # its serial chain isn't head-of-line blocked on DVE
            for st in range(2 * GSTS):
                loadA(st)
            gens = [genA(st) for st in range(GSTS)]
            drain(gens, 2)
            gens = [sk.pop(0)]
            drain(gens, SK_ITERS + 3)
            # remaining loads/cm/sketches run alongside phase C
            live = [genA(st) for st in range(GSTS, 2 * GSTS)]
            for st in range(2 * GSTS, N_ST):
                loadA(st)
            live += [genA(st) for st in range(2 * GSTS, N_ST)]
            live.append(sk.pop(1))
            live.append(sk.pop(2))
            live.append(sk.pop(3))
            nprs = N_ST // 2
            pending = list(range(nprs))
            cs = []
            prog = {}
            STAG = 12
            while pending or cs or live:
                if pending and (not cs or
                                (len(cs) == 1 and prog[id(cs[0])] >= STAG)):
                    g = c_gens.pop(pending.pop(0))
                    cs.append(g)
                    prog[id(g)] = 0
                drain(live, 1)
                for g in list(cs):
                    try:
                        next(g)
                        prog[id(g)] += 1
                    except StopIteration:
                        cs.remove(g)

    orig_tables = bacc.get_activation_tables

    def _lnexp_only(arch):
        return {k: (vv if k == "natural_log_exp_and_others" else set())
                for k, vv in orig_tables(arch).items()}

    bacc.get_activation_tables = _lnexp_only
    try:
        nc.finalize()
    finally:
        bacc.get_activation_tables = orig_tables
    return nc


_NC_CACHE = None


def _get_nc():
    global _NC_CACHE
    if _NC_CACHE is None:
        _NC_CACHE = _build()
    return _NC_CACHE


def _make_cst(al, core):
    """Per-core constant table [128, CST_W] fp32."""
    c = np.zeros(CST_W, np.float64)
    for st in range(N_ST):
        h = (core * BLOCKS + st // (Q // ST_ROWS)) % H
        s = al[h] - 1.0
        p = 1.0 / s
        g, gl = st // GSTS, st % GSTS
        base = g * GW + gl * R * NCH
        c[base:base + R * NCH] = p
        c[1024 + st * R:1024 + st * R + R] = 1.0 / s
        c[1088 + st * R:1088 + st * R + R] = 1.0 / (p * s)
        c[1152 + st * R:1152 + st * R + R] = p * s
        c[1216 + st * R:1216 + st * R + R] = (1.0 / K) ** s
        dm0 = 1.0 - (1.0 / K) ** s
        for i in range(6):
            c[1280 + i * NC + st * R:1280 + i * NC + st * R + R] = \
                dm0 * (0.5 ** (i + 1))
        c[1664 + st] = s
        c[1680 + st] = p
        c[1696 + st] = p - 1.0
    return np.tile(c.astype(np.float32)[None, :], (128, 1))


def kernel(att_scores: np.ndarray, alpha: np.ndarray) -> np.ndarray:
    X = np.ascontiguousarray(np.asarray(att_scores, dtype=np.float32))
    X = X.reshape(B * H, Q, K)
    al = np.asarray(alpha, dtype=np.float64).reshape(H)

    nc = _get_nc()
    in_maps = []
    for c in range(NCORES):
        xc = np.ascontiguousarray(
            X[c * BLOCKS:(c + 1) * BLOCKS].reshape(BLOCKS * Q, K))
        in_maps.append({"x": xc, "cst": _make_cst(al, c)})

    res = run_bass_kernel_spmd(nc, in_maps, core_ids=list(range(NCORES)))
    global LAST_RESULT
    LAST_RESULT = res
    outs = [np.asarray(res.results[c]["y"]) for c in range(NCORES)]
    return np.concatenate(outs, axis=0).reshape(B, H, Q, K).astype(np.float32)
